# revision 18
# baseline (speedup 1.0000x reference)
"""Distributed Trainium2 kernel for nn_Encoder_88502096101469.

8-core SPMD layout (one NEFF, per-core data):
- Activations live TRANSPOSED in SBUF as batch-halves: X^T_b (512 feat x
  256 cols) where cols = batch-b rows [256c, 256c+256) for core c.
- Core c owns attention head h=c for BOTH batches. The torch-faithful
  "raw reshape" of (b, h, t, dv) -> (b, t, h*dv) maps head h's output to
  Z rows [256h, 256h+256) per batch, which is exactly core c's resident
  row range -> no post-attention exchange needed.
- Per batch, one 8-way AllToAll exchanges Q^T/K^T slices (64 head rows x
  256 local cols, bf16) and V natural slices (256 rows x 64 head cols).
- The whole layer is pipelined by batch-half: projections, pack, A2A,
  O-projection and LayerNorm of one half overlap attention of the other.
- v3 vs baseline:
  * All matmuls bf16 (projections were fp32 HIGH mode = 2 cyc/col with
    3x slower LDWEIGHTS).
  * Score matmuls run as concurrent row-tiled pairs (K=64 contraction
    only fills half the PE rows; two key-chunks execute simultaneously
    on row groups 0-1 / 2-3 via qh/kh replicated to partitions 64-127).
  * Softmax exp split between ScalarE (table exp) and VectorE
    (Schraudolph bf16 bit-trick: bits16 = int16(A*s + B)) so the
    2048x2048 exp stream is not serialized on one engine.
  * PSUM tags decoupled: next-layer Q/K/V projections never share a
    psum buffer with O-proj/LN of the current layer, so the QKV->pack->
    AllToAll chain for layer l+1 overlaps attention of layer l.
  * attV accumulator evacuated to SBUF immediately (frees the single
    o-psum bank; recip/normalize run off the SBUF copy).
  * A2A pack/unpack DMAs consolidated (9 big DMAs instead of 29).
  * The residual / LayerNorm stream stays in f32 (bf16 storage there
    compounds to ~5e-2 max error over 6 layers); projections read a
    bf16 copy made once per LN.
- Softmax skips max-subtraction (logits >= 0, O(1)); the denominator
  comes from a ones-column appended to V (lhsT M=65); exp folds the 1/8.
"""
import numpy as np
import ml_dtypes

import concourse.bass as bass
import concourse.bacc as bacc
import concourse.tile as tile
from concourse import mybir
from concourse import bass_utils

NCORES = 8
DIM = 512
HALF = 256          # per-core cols per batch
NITER = 3           # LAYERS + 1
LN_EPS = 1e-5

F32 = mybir.dt.float32
BF16 = mybir.dt.bfloat16
I16 = mybir.dt.int16
I32 = mybir.dt.int32
AF = mybir.ActivationFunctionType
OP = mybir.AluOpType

# Schraudolph bf16 exp: bits16 = int16(A*s_raw + B) where s_raw = q.k
# (the 1/8 logit scale is folded into A).  ~3% max relative error; the
# softmax ratio and 2048-key averaging wash it out (validated in numpy
# and on hardware: 3.4% elementwise, end-to-end insensitive).
EXP_A = (128.0 / np.log(2.0)) * 0.125
EXP_B = 16256.0 - 128.0 * 0.0465

# A2A per-batch shard layout (flat bf16 words per (src,dst) pair):
#   [0:16384)      Q^T slice  (64 of-rows, 256 cols)
#   [16384:32768)  K^T slice  (64 of-rows, 256 cols)
#   [32768:49152)  V slice    (2 t-chunks, 128 rows, 64 fv-cols)
SHARD = 49152


def _build_graph(nc):
    xt_in = nc.dram_tensor("xt", [DIM, 2 * HALF], F32, kind="ExternalInput").ap()
    wq_in = nc.dram_tensor("wq", [DIM, DIM], BF16, kind="ExternalInput").ap()
    wk_in = nc.dram_tensor("wk", [DIM, DIM], BF16, kind="ExternalInput").ap()
    wv_in = nc.dram_tensor("wv", [DIM, DIM], BF16, kind="ExternalInput").ap()
    wo_in = nc.dram_tensor("wo", [DIM, DIM], BF16, kind="ExternalInput").ap()
    bq_in = nc.dram_tensor("bq", [128, 4], F32, kind="ExternalInput").ap()
    bk_in = nc.dram_tensor("bk", [128, 4], F32, kind="ExternalInput").ap()
    bo_in = nc.dram_tensor("bo", [128, 4], F32, kind="ExternalInput").ap()
    bv_in = nc.dram_tensor("bv", [1, DIM], BF16, kind="ExternalInput").ap()
    lng_in = nc.dram_tensor("lng", [128, 4], F32, kind="ExternalInput").ap()
    lnb_in = nc.dram_tensor("lnb", [128, 4], F32, kind="ExternalInput").ap()
    ones_in = nc.dram_tensor("ones", [128, 128], BF16, kind="ExternalInput").ap()
    invd_in = nc.dram_tensor("invd", [128, 2], BF16, kind="ExternalInput").ap()
    out_d = nc.dram_tensor("out", [DIM, 2 * HALF], F32, kind="ExternalOutput").ap()

    groups = [list(range(NCORES))]

    from contextlib import ExitStack
    with tile.TileContext(nc) as tc, ExitStack() as ctx:
        const = ctx.enter_context(tc.tile_pool(name="const", bufs=1))
        act = ctx.enter_context(tc.tile_pool(name="act", bufs=1))
        qkv = ctx.enter_context(tc.tile_pool(name="qkv", bufs=1))
        gath = ctx.enter_context(tc.tile_pool(name="gath", bufs=2))
        epool = ctx.enter_context(tc.tile_pool(name="epool", bufs=4))
        small = ctx.enter_context(tc.tile_pool(name="small", bufs=1))
        dram = ctx.enter_context(tc.tile_pool(name="dram", bufs=1, space="DRAM"))
        s_psum = ctx.enter_context(tc.tile_pool(name="s_psum", bufs=2, space="PSUM"))
        o_psum = ctx.enter_context(tc.tile_pool(name="o_psum", bufs=1, space="PSUM"))
        p_psum = ctx.enter_context(tc.tile_pool(name="p_psum", bufs=2, space="PSUM"))
        po_psum = ctx.enter_context(tc.tile_pool(name="po_psum", bufs=1, space="PSUM"))
        if True:
            # ---- constants to SBUF ----
            def load_w(ap_in, nm):
                t = const.tile([128, 4, DIM], BF16, name=nm, tag=nm)
                nc.sync.dma_start(out=t, in_=ap_in.rearrange("(c p) f -> p c f", p=128))
                return t

            wq, wk, wv, wo = (load_w(wq_in, "wqt"), load_w(wk_in, "wkt"),
                              load_w(wv_in, "wvt"), load_w(wo_in, "wot"))
            bq = const.tile([128, 4], F32)
            bk = const.tile([128, 4], F32)
            bo = const.tile([128, 4], F32)
            lng = const.tile([128, 4], F32)
            lnb = const.tile([128, 4], F32)
            for t, a in ((bq, bq_in), (bk, bk_in), (bo, bo_in), (lng, lng_in), (lnb, lnb_in)):
                nc.sync.dma_start(out=t, in_=a)
            bv = const.tile([1, DIM], BF16)
            nc.sync.dma_start(out=bv, in_=bv_in)
            ones = const.tile([128, 128], BF16)
            nc.sync.dma_start(out=ones, in_=ones_in)
            invd = const.tile([128, 2], BF16)
            nc.sync.dma_start(out=invd, in_=invd_in)
            magic = const.tile([1, HALF], I32)
            nc.vector.memset(magic, 0x5F3759DF)
            onesf = const.tile([1, 128], F32)
            nc.vector.memset(onesf, 1.0)

            # initial activation, as halves: f32 residual + bf16 proj copy
            x0h, x0b = [], []
            for b in range(2):
                xb = act.tile([128, 4, HALF], F32, tag=f"x0h{b}", name=f"x0h{b}")
                nc.sync.dma_start(
                    out=xb,
                    in_=xt_in.rearrange("(c p) f -> p c f", p=128)[:, :, 256 * b:256 * (b + 1)])
                x0h.append(xb)
                xbb = act.tile([128, 4, HALF], BF16, tag=f"x0b{b}", name=f"x0b{b}")
                nc.vector.tensor_copy(xbb, xb)
                x0b.append(xbb)

            # DRAM bounce buffers
            sendb = [dram.tile([NCORES, SHARD], BF16, tag=f"send{b}",
                               name=f"send{b}") for b in range(2)]
            recvb = [dram.tile([NCORES, SHARD], BF16, tag=f"recv{b}",
                               name=f"recv{b}") for b in range(2)]
            rs_d = dram.tile([1, 512], F32, tag="rs_d", name="rs_d", bufs=2)
            stat_d = [dram.tile([2, HALF], F32, tag=f"stat{b}",
                                name=f"stat{b}", bufs=2) for b in range(2)]

            def proj_T_half(x_b, w, btile, tag, pool, odt=BF16):
                """(128,4,HALF) <- relu(w^T x_b + bias), transposed output."""
                out = qkv.tile([128, 4, HALF], odt, tag=tag, name=tag)
                for pair in range(2):
                    ps = pool.tile([128, 2, HALF], F32, tag="p", name=f"ps_{tag}")
                    for i in range(2):
                        ofc = 2 * pair + i
                        for ifc in range(4):
                            nc.tensor.matmul(
                                ps[:, i, :],
                                w[:, ifc, 128 * ofc:128 * (ofc + 1)],
                                x_b[:, ifc, :],
                                start=(ifc == 0), stop=(ifc == 3))
                        nc.vector.tensor_scalar(
                            out=out[:, ofc, :], in0=ps[:, i, :],
                            scalar1=btile[:, ofc:ofc + 1], scalar2=0.0,
                            op0=OP.add, op1=OP.max)
                return out

            def proj_V_half(x_b, tag):
                """(128,2,DIM) bf16 <- relu(x_b^T wv + bv), natural layout."""
                out = qkv.tile([128, 2, DIM], BF16, tag=tag, name=tag)
                for tch in range(2):
                    ps = p_psum.tile([128, DIM], F32, tag="p", name=f"ps_{tag}{tch}")
                    for ifc in range(4):
                        nc.tensor.matmul(
                            ps,
                            x_b[:, ifc, 128 * tch:128 * (tch + 1)],
                            wv[:, ifc, :],
                            start=(ifc == 0), stop=False)
                    nc.tensor.matmul(
                        ps, ones[0:1, :], bv, start=False, stop=True)
                    nc.vector.tensor_scalar(
                        out=out[:, tch, :], in0=ps,
                        scalar1=0.0, scalar2=None, op0=OP.max)
                return out

            def exchange_half(b, qt_b, kt_b, v_b):
                sb, rb = sendb[b], recvb[b]
                # pack Q, K: 2 DMAs each (one per 64-partition half)
                for base, src in ((0, qt_b), (16384, kt_b)):
                    seg = sb[:, base:base + 16384].rearrange(
                        "(cq two) (r c) -> two r cq c", two=2, c=256)
                    for two in range(2):
                        nc.sync.dma_start(
                            out=seg[two],
                            in_=src[64 * two:64 * (two + 1), :, :])
                # pack V: 2 DMAs (one per t-chunk; 4-dim APs don't balance)
                for tc in range(2):
                    nc.sync.dma_start(
                        out=sb[:, 32768 + 8192 * tc:32768 + 8192 * (tc + 1)]
                            .rearrange("d (p j) -> p d j", p=128),
                        in_=v_b[:, tc, :].rearrange("p (d j) -> p d j", j=64))
                nc.gpsimd.collective_compute(
                    "AllToAll", OP.bypass, replica_groups=groups,
                    ins=[sb.opt()], outs=[rb.opt()])
                # unpack with 64->128 replication for row-tiled score matmuls
                qh = gath.tile([128, 8, 256], BF16, tag=f"qh{b}", name=f"qh{b}")
                kh = gath.tile([128, 8, 256], BF16, tag=f"kh{b}", name=f"kh{b}")
                for dst, base in ((qh, 0), (kh, 16384)):
                    for hh in range(2):
                        nc.sync.dma_start(
                            out=dst[64 * hh:64 * (hh + 1), :, :],
                            in_=rb[:, base:base + 16384]
                                .rearrange("s (r c) -> r s c", r=64))
                vh = gath.tile([128, 16, 65], BF16, tag=f"vh{b}", name=f"vh{b}")
                for tc2 in range(2):
                    nc.sync.dma_start(
                        out=vh[:, tc2::2, 0:64],
                        in_=rb[:, 32768 + 8192 * tc2:32768 + 8192 * (tc2 + 1)]
                            .rearrange("s (p j) -> p s j", p=128))
                nc.vector.memset(vh[:, :, 64:65], 1.0)
                return qh, kh, vh

            def attention_half(b, qh, kh, vh, fillers=()):
                """(128,4,HALF) bf16 Z^T for batch b (local Z rows).

                `fillers` are thunks (next-layer projection chunks) called
                one per score group so their PE matmuls land BETWEEN
                attention matmuls in the engine FIFOs -- the only way to
                fill the PE during the exp-bound attention stream.
                The per-column normalize (recip + z-scale) is deferred to
                the end so its DMA bounce never head-of-line-blocks the
                DVE queue between exp groups.
                """
                fill_iter = iter(fillers)
                z = qkv.tile([128, 4, HALF], BF16, tag=f"z{b}", name=f"z{b}")
                cols = []
                for j in range(4):
                    ops = o_psum.tile([65, 512], F32, tag="o", name=f"ops{b}{j}")
                    dve_groups = (6, 7) if j % 2 == 0 else (7,)
                    for g in range(8):
                        sps = s_psum.tile([128, 2, 512], F32, tag="s",
                                          name=f"sps{b}{j}{g}")
                        for u in range(2):
                            k = 2 * g + u
                            nc.tensor.matmul(
                                sps[:, u, :],
                                kh[64 * u:64 * (u + 1), k // 2,
                                   128 * (k % 2):128 * (k % 2 + 1)],
                                qh[64 * u:64 * (u + 1), 2 * j:2 * j + 2, :],
                                start=True, stop=True)
                        e = epool.tile([128, 2, 512], BF16, tag="e", name=f"e{b}{j}{g}")
                        if g in dve_groups:
                            nc.vector.tensor_scalar(
                                out=e.bitcast(I16), in0=sps,
                                scalar1=float(EXP_A), scalar2=float(EXP_B),
                                op0=OP.mult, op1=OP.add)
                        else:
                            nc.scalar.activation(e, sps, AF.Exp, scale=0.125)
                        for u in range(2):
                            k = 2 * g + u
                            nc.tensor.matmul(
                                ops, vh[:, k, :], e[:, u, :],
                                start=(k == 0), stop=(k == 15))
                        f = next(fill_iter, None)
                        if f is not None:
                            f()
                    # evacuate the single o-psum bank immediately; bounce
                    # the denominator row out for the 64-way broadcast
                    oc = small.tile([65, 512], F32, tag="oc", bufs=4, name="oc")
                    nc.vector.tensor_copy(oc, ops)
                    nc.sync.dma_start(out=rs_d, in_=oc[64:65, :])
                    dvec = small.tile([64, 512], F32, tag="dvec", bufs=4, name="dvec")
                    nc.sync.dma_start(
                        out=dvec, in_=rs_d.partition_broadcast(64)[:, 0, :])
                    cols.append((oc, dvec))
                for f in fill_iter:
                    f()
                for j, (oc, dvec) in enumerate(cols):
                    rrep = small.tile([64, 512], F32, tag="rrep", bufs=2, name="rrep")
                    nc.vector.reciprocal_approx_fast(rrep, dvec)
                    o_v = oc[0:64, :].rearrange("f (r s) -> f s r", s=8)
                    r_v = rrep.rearrange("f (r s) -> f s r", s=8)
                    for q in range(2):
                        nc.vector.tensor_tensor(
                            out=z[64 * q:64 * (q + 1), :, 64 * j:64 * (j + 1)],
                            in0=o_v[:, q::2, :],
                            in1=r_v[:, q::2, :],
                            op=OP.mult)
                return z

            def ln_half(x_b, b, resid=None):
                """LN over features (partitions) on one f32 batch-half.

                Returns (bf16_out, f32_out_or_None); f32 only for residual
                layers (the residual stream must stay f32).
                """
                if resid is not None:
                    xr = act.tile([128, 4, HALF], F32, tag=f"xr{b}", name=f"xr{b}")
                    nc.vector.tensor_tensor(out=xr, in0=x_b, in1=resid, op=OP.add)
                    x_b = xr
                # bf16 copies for the stat matmuls
                xb16 = act.tile([128, 4, HALF], BF16, tag=f"xb{b}", name=f"xb{b}")
                nc.vector.tensor_copy(xb16, x_b)
                x2 = act.tile([128, 4, HALF], BF16, tag=f"x2{b}", name=f"x2{b}")
                nc.vector.tensor_tensor(out=x2, in0=xb16, in1=xb16, op=OP.mult)
                # mean and mean-of-squares via ones/512 matmuls (partition sums)
                mu_ps = po_psum.tile([1, HALF], F32, tag="p", name=f"lnmu{b}")
                for ifc in range(4):
                    nc.tensor.matmul(mu_ps, invd[:, 0:1], xb16[:, ifc, :],
                                     start=(ifc == 0), stop=(ifc == 3))
                mu = small.tile([1, HALF], F32, tag="mu", bufs=2, name="mu")
                nc.vector.tensor_copy(mu, mu_ps)
                m2_ps = po_psum.tile([1, HALF], F32, tag="p", name=f"lnm2{b}")
                for ifc in range(4):
                    nc.tensor.matmul(m2_ps, invd[:, 1:2], x2[:, ifc, :],
                                     start=(ifc == 0), stop=(ifc == 3))
                # var + eps = ex2 - mu^2 + eps
                m2 = small.tile([1, HALF], F32, tag="m2", bufs=2, name="m2")
                nc.vector.tensor_tensor(out=m2, in0=mu, in1=mu, op=OP.mult)
                ex2 = small.tile([1, HALF], F32, tag="ex2", bufs=2, name="ex2")
                nc.vector.tensor_tensor(out=ex2, in0=m2_ps, in1=m2, op=OP.subtract)
                nc.vector.tensor_scalar(out=ex2, in0=ex2, scalar1=LN_EPS,
                                        scalar2=None, op0=OP.add)
                # rstd = rsqrt(var+eps) on DVE: bit-trick seed + 2 Newton steps
                sd = small.tile([1, HALF], F32, tag="sd", bufs=2, name="sd")
                sdi = sd.bitcast(I32)
                nc.vector.tensor_scalar(out=sdi, in0=ex2.bitcast(I32), scalar1=1,
                                        scalar2=None, op0=OP.logical_shift_right)
                nc.vector.tensor_tensor(out=sdi, in0=magic, in1=sdi, op=OP.subtract)
                for _ in range(2):
                    nc.vector.tensor_tensor(out=m2, in0=ex2, in1=sd, op=OP.mult)
                    nc.vector.tensor_tensor(out=m2, in0=m2, in1=sd, op=OP.mult)
                    nc.vector.tensor_scalar(out=m2, in0=m2, scalar1=-0.5,
                                            scalar2=1.5, op0=OP.mult, op1=OP.add)
                    nc.vector.tensor_tensor(out=sd, in0=sd, in1=m2, op=OP.mult)
                nc.vector.tensor_tensor(out=mu, in0=mu, in1=sd, op=OP.mult)
                # broadcast rstd & mu*rstd across partitions via a K=1
                # outer-product matmul (no DRAM bounce -> no DVE FIFO
                # head-of-line stall waiting on a DMA round trip)
                srep = po_psum.tile([128, 2, HALF], F32, tag="p",
                                    name=f"srep{b}")
                nc.tensor.matmul(srep[:, 0, :], onesf[0:1, :], sd,
                                 start=True, stop=True)
                nc.tensor.matmul(srep[:, 1, :], onesf[0:1, :], mu,
                                 start=True, stop=True)
                outb = act.tile([128, 4, HALF], BF16, tag=f"lnb{b}", bufs=2,
                                name=f"lnb{b}")
                outf = None
                if resid is not None:
                    outf = act.tile([128, 4, HALF], F32, tag=f"lnf{b}", bufs=2,
                                    name=f"lnf{b}")
                for ifc in range(4):
                    t1 = small.tile([128, HALF], F32, tag=f"t1{b}", bufs=2,
                                    name=f"t1{b}")
                    nc.vector.tensor_tensor(out=t1, in0=x_b[:, ifc, :],
                                            in1=srep[:, 0, :], op=OP.mult)
                    nc.vector.tensor_tensor(out=t1, in0=t1, in1=srep[:, 1, :],
                                            op=OP.subtract)
                    nc.vector.tensor_scalar(
                        out=outb[:, ifc, :], in0=t1,
                        scalar1=lng[:, ifc:ifc + 1], scalar2=lnb[:, ifc:ifc + 1],
                        op0=OP.mult, op1=OP.add)
                    if outf is not None:
                        nc.vector.tensor_scalar(
                            out=outf[:, ifc, :], in0=t1,
                            scalar1=lng[:, ifc:ifc + 1], scalar2=lnb[:, ifc:ifc + 1],
                            op0=OP.mult, op1=OP.add)
                return outb, outf

            def qkv_exchange(b, x_b16):
                qt_b = proj_T_half(x_b16, wq, bq, f"qt{b}", p_psum)
                kt_b = proj_T_half(x_b16, wk, bk, f"kt{b}", p_psum)
                v_b = proj_V_half(x_b16, f"v{b}")
                return exchange_half(b, qt_b, kt_b, v_b)

            def proj_fillers(b, x_b16, key, pk_store):
                """Thunks that together emit QKV proj + pack + A2A + unpack
                for (layer, half) = key, sliced so one thunk fits in one
                attention score-group's PE shadow."""
                tiles = {}

                def out_tile(tag, shape):
                    if tag not in tiles:
                        tiles[tag] = qkv.tile(shape, BF16, tag=tag, name=tag)
                    return tiles[tag]

                def t_pair(w, btile, tag, pair):
                    def run():
                        out = out_tile(tag, [128, 4, HALF])
                        ps = p_psum.tile([128, 2, HALF], F32, tag="p",
                                         name=f"ps_{tag}{pair}")
                        for i in range(2):
                            ofc = 2 * pair + i
                            for ifc in range(4):
                                nc.tensor.matmul(
                                    ps[:, i, :],
                                    w[:, ifc, 128 * ofc:128 * (ofc + 1)],
                                    x_b16[:, ifc, :],
                                    start=(ifc == 0), stop=(ifc == 3))
                            nc.vector.tensor_scalar(
                                out=out[:, ofc, :], in0=ps[:, i, :],
                                scalar1=btile[:, ofc:ofc + 1], scalar2=0.0,
                                op0=OP.add, op1=OP.max)
                    return run

                def v_chunk(tch):
                    def run():
                        out = out_tile(f"v{b}", [128, 2, DIM])
                        ps = p_psum.tile([128, DIM], F32, tag="p",
                                         name=f"ps_v{b}{tch}")
                        for ifc in range(4):
                            nc.tensor.matmul(
                                ps,
                                x_b16[:, ifc, 128 * tch:128 * (tch + 1)],
                                wv[:, ifc, :],
                                start=(ifc == 0), stop=False)
                        nc.tensor.matmul(
                            ps, ones[0:1, :], bv, start=False, stop=True)
                        nc.vector.tensor_scalar(
                            out=out[:, tch, :], in0=ps,
                            scalar1=0.0, scalar2=None, op0=OP.max)
                    return run

                def xchg():
                    pk_store[key] = exchange_half(
                        b, tiles[f"qt{b}"], tiles[f"kt{b}"], tiles[f"v{b}"])

                return [t_pair(wq, bq, f"qt{b}", 0), t_pair(wq, bq, f"qt{b}", 1),
                        t_pair(wk, bk, f"kt{b}", 0), t_pair(wk, bk, f"kt{b}", 1),
                        v_chunk(0), v_chunk(1), xchg]

            # Flat (layer, half) software pipeline.  During attention of
            # one half, the thunks for the NEXT pending (layer, half)'s
            # QKV+A2A are interleaved into the instruction stream, so the
            # PE fills its exp-wait holes with projection matmuls and the
            # A2A rides behind the attention phase.
            NLAYERS = 2 * NITER
            resid_f = list(x0h)            # f32 residual stream (per half)
            pk = {}
            pk[(0, 0)] = qkv_exchange(0, x0b[0])
            pk[(0, 1)] = qkv_exchange(1, x0b[1])
            lnb_store = {}
            final_f = [None, None]
            for m in range(NLAYERS):
                for b in range(2):
                    tgt = (m, 1) if b == 0 else (m + 1, 0)
                    src = lnb_store.get((tgt[0] - 1, tgt[1]))
                    fillers = ()
                    if tgt not in pk and tgt[0] < NLAYERS and src is not None:
                        fillers = proj_fillers(tgt[1], src, tgt, pk)
                    z_b = attention_half(b, *pk[(m, b)], fillers=fillers)
                    y_b = proj_T_half(z_b, wo, bo, f"y{b}", po_psum, odt=F32)
                    if m % 2 == 1:
                        ob, of = ln_half(y_b, b, resid=resid_f[b])
                        resid_f[b] = of
                        final_f[b] = of
                    else:
                        ob, _ = ln_half(y_b, b)
                    lnb_store[(m, b)] = ob

            for b in range(2):
                nc.sync.dma_start(
                    out=out_d.rearrange("(c p) f -> p c f", p=128)[:, :, 256 * b:256 * (b + 1)],
                    in_=final_f[b])
    return nc


_NC_CACHE = None


def _get_nc():
    global _NC_CACHE
    if _NC_CACHE is None:
        nc = bacc.Bacc("TRN2", target_bir_lowering=False, debug=False,
                       num_devices=NCORES)
        _build_graph(nc)
        nc.compile()
        _NC_CACHE = nc
    return _NC_CACHE


def kernel(encoder_inputs, Wq, bq, Wk, bk, Wv, bv, Wo, bo, ln_g, ln_b,
           _trace=False, _trace_kwargs=None):
    BF = ml_dtypes.bfloat16
    x = np.asarray(encoder_inputs, dtype=np.float32)
    consts = {
        "wq": np.ascontiguousarray(np.asarray(Wq, np.float32).astype(BF)),
        "wk": np.ascontiguousarray(np.asarray(Wk, np.float32).astype(BF)),
        "wv": np.ascontiguousarray(np.asarray(Wv, np.float32).astype(BF)),
        "wo": np.ascontiguousarray(np.asarray(Wo, np.float32).astype(BF)),
        "bq": np.ascontiguousarray(np.asarray(bq, np.float32).reshape(4, 128).T),
        "bk": np.ascontiguousarray(np.asarray(bk, np.float32).reshape(4, 128).T),
        "bo": np.ascontiguousarray(np.asarray(bo, np.float32).reshape(4, 128).T),
        "bv": np.asarray(bv, np.float32).astype(BF).reshape(1, DIM),
        "lng": np.ascontiguousarray(np.asarray(ln_g, np.float32).reshape(4, 128).T),
        "lnb": np.ascontiguousarray(np.asarray(ln_b, np.float32).reshape(4, 128).T),
        "ones": np.ones((128, 128), BF),
        "invd": np.full((128, 2), 1.0 / DIM, BF),
    }
    in_maps = []
    for c in range(NCORES):
        xt = np.concatenate([x[0, 256 * c:256 * (c + 1)].T,
                             x[1, 256 * c:256 * (c + 1)].T], axis=1)
        in_maps.append({"xt": np.ascontiguousarray(xt), **consts})

    nc = _get_nc()
    res = bass_utils.run_bass_kernel_spmd(
        nc, in_maps, core_ids=list(range(NCORES)),
        trace=_trace, **(_trace_kwargs or {}))

    out = np.zeros((2, 2048, DIM), np.float32)
    for c in range(NCORES):
        r = res.results[c]["out"]
        out[0, 256 * c:256 * (c + 1)] = r[:, :256].T
        out[1, 256 * c:256 * (c + 1)] = r[:, 256:].T
    if _trace:
        kernel._last_results = res
    return out


# revision 22
# speedup vs baseline: 1.0163x; 1.0163x over previous
"""Distributed Trainium2 kernel for nn_Encoder_88502096101469.

8-core SPMD layout (one NEFF, per-core data):
- Activations live TRANSPOSED in SBUF as batch-halves: X^T_b (512 feat x
  256 cols) where cols = batch-b rows [256c, 256c+256) for core c.
- Core c owns attention head h=c for BOTH batches. The torch-faithful
  "raw reshape" of (b, h, t, dv) -> (b, t, h*dv) maps head h's output to
  Z rows [256h, 256h+256) per batch, which is exactly core c's resident
  row range -> no post-attention exchange needed.
- Per batch, one 8-way AllToAll exchanges Q^T/K^T slices (64 head rows x
  256 local cols, bf16) and V natural slices (256 rows x 64 head cols).
- The whole layer is pipelined by batch-half: projections, pack, A2A,
  O-projection and LayerNorm of one half overlap attention of the other.
- v3 vs baseline:
  * All matmuls bf16 (projections were fp32 HIGH mode = 2 cyc/col with
    3x slower LDWEIGHTS).
  * Score matmuls run as concurrent row-tiled pairs (K=64 contraction
    only fills half the PE rows; two key-chunks execute simultaneously
    on row groups 0-1 / 2-3 via qh/kh replicated to partitions 64-127).
  * Softmax exp split between ScalarE (table exp) and VectorE
    (Schraudolph bf16 bit-trick: bits16 = int16(A*s + B)) so the
    2048x2048 exp stream is not serialized on one engine.
  * PSUM tags decoupled: next-layer Q/K/V projections never share a
    psum buffer with O-proj/LN of the current layer, so the QKV->pack->
    AllToAll chain for layer l+1 overlaps attention of layer l.
  * attV accumulator evacuated to SBUF immediately (frees the single
    o-psum bank; recip/normalize run off the SBUF copy).
  * A2A pack/unpack DMAs consolidated (9 big DMAs instead of 29).
  * The residual / LayerNorm stream stays in f32 (bf16 storage there
    compounds to ~5e-2 max error over 6 layers); projections read a
    bf16 copy made once per LN.
- Softmax skips max-subtraction (logits >= 0, O(1)); the denominator
  comes from a ones-column appended to V (lhsT M=65); exp folds the 1/8.
"""
import numpy as np
import ml_dtypes

import concourse.bass as bass
import concourse.bacc as bacc
import concourse.tile as tile
from concourse import mybir
from concourse import bass_utils

NCORES = 8
DIM = 512
HALF = 256          # per-core cols per batch
NITER = 3           # LAYERS + 1
LN_EPS = 1e-5

F32 = mybir.dt.float32
BF16 = mybir.dt.bfloat16
I16 = mybir.dt.int16
I32 = mybir.dt.int32
AF = mybir.ActivationFunctionType
OP = mybir.AluOpType

# Schraudolph bf16 exp: bits16 = int16(A*s_raw + B) where s_raw = q.k
# (the 1/8 logit scale is folded into A).  ~3% max relative error; the
# softmax ratio and 2048-key averaging wash it out (validated in numpy
# and on hardware: 3.4% elementwise, end-to-end insensitive).
EXP_A = (128.0 / np.log(2.0)) * 0.125
EXP_B = 16256.0 - 128.0 * 0.0465

# A2A per-batch shard layout (flat bf16 words per (src,dst) pair):
#   [0:16384)      Q^T slice  (64 of-rows, 256 cols)
#   [16384:32768)  K^T slice  (64 of-rows, 256 cols)
#   [32768:49152)  V slice    (2 t-chunks, 128 rows, 64 fv-cols)
SHARD = 49152


def _build_graph(nc):
    xt_in = nc.dram_tensor("xt", [DIM, 2 * HALF], F32, kind="ExternalInput").ap()
    wq_in = nc.dram_tensor("wq", [DIM, DIM], BF16, kind="ExternalInput").ap()
    wk_in = nc.dram_tensor("wk", [DIM, DIM], BF16, kind="ExternalInput").ap()
    wv_in = nc.dram_tensor("wv", [DIM, DIM], BF16, kind="ExternalInput").ap()
    wo_in = nc.dram_tensor("wo", [DIM, DIM], BF16, kind="ExternalInput").ap()
    bq_in = nc.dram_tensor("bq", [128, 4], F32, kind="ExternalInput").ap()
    bk_in = nc.dram_tensor("bk", [128, 4], F32, kind="ExternalInput").ap()
    bo_in = nc.dram_tensor("bo", [128, 4], F32, kind="ExternalInput").ap()
    bv_in = nc.dram_tensor("bv", [1, DIM], BF16, kind="ExternalInput").ap()
    lng_in = nc.dram_tensor("lng", [128, 4], F32, kind="ExternalInput").ap()
    lnb_in = nc.dram_tensor("lnb", [128, 4], F32, kind="ExternalInput").ap()
    ones_in = nc.dram_tensor("ones", [128, 128], BF16, kind="ExternalInput").ap()
    invd_in = nc.dram_tensor("invd", [128, 2], BF16, kind="ExternalInput").ap()
    out_d = nc.dram_tensor("out", [DIM, 2 * HALF], F32, kind="ExternalOutput").ap()

    groups = [list(range(NCORES))]

    from contextlib import ExitStack
    with tile.TileContext(nc) as tc, ExitStack() as ctx:
        const = ctx.enter_context(tc.tile_pool(name="const", bufs=1))
        act = ctx.enter_context(tc.tile_pool(name="act", bufs=1))
        qkv = ctx.enter_context(tc.tile_pool(name="qkv", bufs=1))
        gath = ctx.enter_context(tc.tile_pool(name="gath", bufs=2))
        epool = ctx.enter_context(tc.tile_pool(name="epool", bufs=3))
        small = ctx.enter_context(tc.tile_pool(name="small", bufs=1))
        dram = ctx.enter_context(tc.tile_pool(name="dram", bufs=1, space="DRAM"))
        s_psum = ctx.enter_context(tc.tile_pool(name="s_psum", bufs=2, space="PSUM"))
        o_psum = ctx.enter_context(tc.tile_pool(name="o_psum", bufs=1, space="PSUM"))
        p_psum = ctx.enter_context(tc.tile_pool(name="p_psum", bufs=2, space="PSUM"))
        po_psum = ctx.enter_context(tc.tile_pool(name="po_psum", bufs=1, space="PSUM"))
        if True:
            # ---- constants to SBUF ----
            def load_w(ap_in, nm):
                t = const.tile([128, 4, DIM], BF16, name=nm, tag=nm)
                nc.sync.dma_start(out=t, in_=ap_in.rearrange("(c p) f -> p c f", p=128))
                return t

            wq, wk, wv, wo = (load_w(wq_in, "wqt"), load_w(wk_in, "wkt"),
                              load_w(wv_in, "wvt"), load_w(wo_in, "wot"))
            bq = const.tile([128, 4], F32)
            bk = const.tile([128, 4], F32)
            bo = const.tile([128, 4], F32)
            lng = const.tile([128, 4], F32)
            lnb = const.tile([128, 4], F32)
            for t, a in ((bq, bq_in), (bk, bk_in), (bo, bo_in), (lng, lng_in), (lnb, lnb_in)):
                nc.sync.dma_start(out=t, in_=a)
            bv = const.tile([1, DIM], BF16)
            nc.sync.dma_start(out=bv, in_=bv_in)
            ones = const.tile([128, 128], BF16)
            nc.sync.dma_start(out=ones, in_=ones_in)
            invd = const.tile([128, 2], BF16)
            nc.sync.dma_start(out=invd, in_=invd_in)
            magic = const.tile([1, HALF], I32)
            nc.vector.memset(magic, 0x5F3759DF)
            onesf = const.tile([1, 128], F32)
            nc.vector.memset(onesf, 1.0)

            # initial activation, as halves: f32 residual + bf16 proj copy
            x0h, x0b = [], []
            for b in range(2):
                xb = act.tile([128, 4, HALF], F32, tag=f"x0h{b}", name=f"x0h{b}")
                nc.sync.dma_start(
                    out=xb,
                    in_=xt_in.rearrange("(c p) f -> p c f", p=128)[:, :, 256 * b:256 * (b + 1)])
                x0h.append(xb)
                xbb = act.tile([128, 4, HALF], BF16, tag=f"x0b{b}", name=f"x0b{b}")
                nc.vector.tensor_copy(xbb, xb)
                x0b.append(xbb)

            # DRAM bounce buffers
            sendb = [dram.tile([NCORES, SHARD], BF16, tag=f"send{b}",
                               name=f"send{b}") for b in range(2)]
            recvb = [dram.tile([NCORES, SHARD], BF16, tag=f"recv{b}",
                               name=f"recv{b}") for b in range(2)]
            rs_d = dram.tile([1, 512], F32, tag="rs_d", name="rs_d", bufs=2)
            stat_d = [dram.tile([2, HALF], F32, tag=f"stat{b}",
                                name=f"stat{b}", bufs=2) for b in range(2)]

            def proj_T_half(x_b, w, btile, tag, pool, odt=BF16):
                """(128,4,HALF) <- relu(w^T x_b + bias), transposed output."""
                out = qkv.tile([128, 4, HALF], odt, tag=tag, name=tag)
                for pair in range(2):
                    ps = pool.tile([128, 2, HALF], F32, tag="p", name=f"ps_{tag}")
                    for i in range(2):
                        ofc = 2 * pair + i
                        for ifc in range(4):
                            nc.tensor.matmul(
                                ps[:, i, :],
                                w[:, ifc, 128 * ofc:128 * (ofc + 1)],
                                x_b[:, ifc, :],
                                start=(ifc == 0), stop=(ifc == 3))
                        nc.vector.tensor_scalar(
                            out=out[:, ofc, :], in0=ps[:, i, :],
                            scalar1=btile[:, ofc:ofc + 1], scalar2=0.0,
                            op0=OP.add, op1=OP.max)
                return out

            def proj_V_half(x_b, tag):
                """(128,2,DIM) bf16 <- relu(x_b^T wv + bv), natural layout."""
                out = qkv.tile([128, 2, DIM], BF16, tag=tag, name=tag)
                for tch in range(2):
                    ps = p_psum.tile([128, DIM], F32, tag="p", name=f"ps_{tag}{tch}")
                    for ifc in range(4):
                        nc.tensor.matmul(
                            ps,
                            x_b[:, ifc, 128 * tch:128 * (tch + 1)],
                            wv[:, ifc, :],
                            start=(ifc == 0), stop=False)
                    nc.tensor.matmul(
                        ps, ones[0:1, :], bv, start=False, stop=True)
                    nc.vector.tensor_scalar(
                        out=out[:, tch, :], in0=ps,
                        scalar1=0.0, scalar2=None, op0=OP.max)
                return out

            def exchange_half(b, qt_b, kt_b, v_b):
                sb, rb = sendb[b], recvb[b]
                # pack Q, K: 2 DMAs each (one per 64-partition half)
                for base, src in ((0, qt_b), (16384, kt_b)):
                    seg = sb[:, base:base + 16384].rearrange(
                        "(cq two) (r c) -> two r cq c", two=2, c=256)
                    for two in range(2):
                        nc.sync.dma_start(
                            out=seg[two],
                            in_=src[64 * two:64 * (two + 1), :, :])
                # pack V: 2 DMAs (one per t-chunk; 4-dim APs don't balance)
                for tc in range(2):
                    nc.sync.dma_start(
                        out=sb[:, 32768 + 8192 * tc:32768 + 8192 * (tc + 1)]
                            .rearrange("d (p j) -> p d j", p=128),
                        in_=v_b[:, tc, :].rearrange("p (d j) -> p d j", j=64))
                nc.gpsimd.collective_compute(
                    "AllToAll", OP.bypass, replica_groups=groups,
                    ins=[sb.opt()], outs=[rb.opt()])
                # unpack with 64->128 replication for row-tiled score matmuls
                qh = gath.tile([128, 8, 256], BF16, tag=f"qh{b}", name=f"qh{b}")
                kh = gath.tile([128, 8, 256], BF16, tag=f"kh{b}", name=f"kh{b}")
                for dst, base in ((qh, 0), (kh, 16384)):
                    for hh in range(2):
                        nc.sync.dma_start(
                            out=dst[64 * hh:64 * (hh + 1), :, :],
                            in_=rb[:, base:base + 16384]
                                .rearrange("s (r c) -> r s c", r=64))
                vh = gath.tile([128, 16, 65], BF16, tag=f"vh{b}", name=f"vh{b}")
                for tc2 in range(2):
                    nc.sync.dma_start(
                        out=vh[:, tc2::2, 0:64],
                        in_=rb[:, 32768 + 8192 * tc2:32768 + 8192 * (tc2 + 1)]
                            .rearrange("s (p j) -> p s j", p=128))
                nc.vector.memset(vh[:, :, 64:65], 1.0)
                return qh, kh, vh

            def attention_half(b, qh, kh, vh, fillers=()):
                """(128,4,HALF) bf16 Z^T for batch b (local Z rows).

                `fillers` are thunks (next-layer projection chunks) called
                one per score group so their PE matmuls land BETWEEN
                attention matmuls in the engine FIFOs -- the only way to
                fill the PE during the exp-bound attention stream.
                The per-column normalize (recip + z-scale) is deferred to
                the end so its DMA bounce never head-of-line-blocks the
                DVE queue between exp groups.
                """
                fill_iter = iter(fillers)
                z = qkv.tile([128, 4, HALF], BF16, tag=f"z{b}", name=f"z{b}")
                cols = []
                for j in range(4):
                    ops = o_psum.tile([65, 512], F32, tag="o", name=f"ops{b}{j}")
                    dve_groups = (2, 5) if j % 2 == 0 else (4,)
                    for g in range(8):
                        sps = s_psum.tile([128, 2, 512], F32, tag="s",
                                          name=f"sps{b}{j}{g}")
                        for u in range(2):
                            k = 2 * g + u
                            nc.tensor.matmul(
                                sps[:, u, :],
                                kh[64 * u:64 * (u + 1), k // 2,
                                   128 * (k % 2):128 * (k % 2 + 1)],
                                qh[64 * u:64 * (u + 1), 2 * j:2 * j + 2, :],
                                start=True, stop=True)
                        e = epool.tile([128, 2, 512], BF16, tag="e", name=f"e{b}{j}{g}")
                        if g in dve_groups:
                            nc.vector.tensor_scalar(
                                out=e.bitcast(I16), in0=sps,
                                scalar1=float(EXP_A), scalar2=float(EXP_B),
                                op0=OP.mult, op1=OP.add)
                        else:
                            nc.scalar.activation(e, sps, AF.Exp, scale=0.125)
                        for u in range(2):
                            k = 2 * g + u
                            nc.tensor.matmul(
                                ops, vh[:, k, :], e[:, u, :],
                                start=(k == 0), stop=(k == 15))
                        f = next(fill_iter, None)
                        if f is not None:
                            f()
                    # evacuate the single o-psum bank immediately; bounce
                    # the denominator row out for the 64-way broadcast
                    oc = small.tile([65, 512], F32, tag="oc", bufs=4, name="oc")
                    nc.vector.tensor_copy(oc, ops)
                    nc.sync.dma_start(out=rs_d, in_=oc[64:65, :])
                    dvec = small.tile([64, 512], F32, tag="dvec", bufs=4, name="dvec")
                    nc.sync.dma_start(
                        out=dvec, in_=rs_d.partition_broadcast(64)[:, 0, :])
                    cols.append((oc, dvec))
                for f in fill_iter:
                    f()
                for j, (oc, dvec) in enumerate(cols):
                    rrep = small.tile([64, 512], F32, tag="rrep", bufs=2, name="rrep")
                    nc.vector.reciprocal_approx_fast(rrep, dvec)
                    o_v = oc[0:64, :].rearrange("f (r s) -> f s r", s=8)
                    r_v = rrep.rearrange("f (r s) -> f s r", s=8)
                    for q in range(2):
                        nc.vector.tensor_tensor(
                            out=z[64 * q:64 * (q + 1), :, 64 * j:64 * (j + 1)],
                            in0=o_v[:, q::2, :],
                            in1=r_v[:, q::2, :],
                            op=OP.mult)
                return z

            def proj_fillers(b, get_x, key, pk_store):
                """Thunks that together emit QKV proj + pack + A2A + unpack
                for (layer, half) = key, sliced so one thunk fits in one
                attention score-group's PE shadow."""
                tiles = {}

                def out_tile(tag, shape):
                    if tag not in tiles:
                        tiles[tag] = qkv.tile(shape, BF16, tag=tag, name=tag)
                    return tiles[tag]

                def t_pair(w, btile, tag, pair):
                    def run():
                        x_b16 = get_x()
                        out = out_tile(tag, [128, 4, HALF])
                        ps = p_psum.tile([128, 2, HALF], F32, tag="p",
                                         name=f"ps_{tag}{pair}")
                        for i in range(2):
                            ofc = 2 * pair + i
                            for ifc in range(4):
                                nc.tensor.matmul(
                                    ps[:, i, :],
                                    w[:, ifc, 128 * ofc:128 * (ofc + 1)],
                                    x_b16[:, ifc, :],
                                    start=(ifc == 0), stop=(ifc == 3))
                            nc.vector.tensor_scalar(
                                out=out[:, ofc, :], in0=ps[:, i, :],
                                scalar1=btile[:, ofc:ofc + 1], scalar2=0.0,
                                op0=OP.add, op1=OP.max)
                    return run

                def v_chunk(tch):
                    def run():
                        x_b16 = get_x()
                        out = out_tile(f"v{b}", [128, 2, DIM])
                        ps = p_psum.tile([128, DIM], F32, tag="p",
                                         name=f"ps_v{b}{tch}")
                        for ifc in range(4):
                            nc.tensor.matmul(
                                ps,
                                x_b16[:, ifc, 128 * tch:128 * (tch + 1)],
                                wv[:, ifc, :],
                                start=(ifc == 0), stop=False)
                        nc.tensor.matmul(
                            ps, ones[0:1, :], bv, start=False, stop=True)
                        nc.vector.tensor_scalar(
                            out=out[:, tch, :], in0=ps,
                            scalar1=0.0, scalar2=None, op0=OP.max)
                    return run

                def xchg():
                    pk_store[key] = exchange_half(
                        b, tiles[f"qt{b}"], tiles[f"kt{b}"], tiles[f"v{b}"])

                return [t_pair(wq, bq, f"qt{b}", 0), t_pair(wq, bq, f"qt{b}", 1),
                        t_pair(wk, bk, f"kt{b}", 0), t_pair(wk, bk, f"kt{b}", 1),
                        v_chunk(0), v_chunk(1), xchg]

            def make_tail(m, b, z_b, resid):
                """Thunks for the post-attention tail of (m, b): O-proj,
                LN stats, variance/rsqrt chain, LN apply.  Consumed as
                fillers inside the NEXT attention phase so these PE ops
                (behind the serial LN DVE chain) never head-of-line-block
                the next attention stream in the engine FIFOs."""
                h = {}

                def t_oproj():
                    h["y"] = proj_T_half(z_b, wo, bo, f"y{b}", po_psum,
                                         odt=F32)

                def t_stats():
                    x_b = h["y"]
                    if resid is not None:
                        xr = act.tile([128, 4, HALF], F32, tag=f"xr{b}",
                                      name=f"xr{b}")
                        nc.vector.tensor_tensor(out=xr, in0=x_b, in1=resid,
                                                op=OP.add)
                        x_b = xr
                    h["x"] = x_b
                    xb16 = act.tile([128, 4, HALF], BF16, tag=f"xb{b}",
                                    name=f"xb{b}")
                    nc.vector.tensor_copy(xb16, x_b)
                    x2 = act.tile([128, 4, HALF], BF16, tag=f"x2{b}",
                                  name=f"x2{b}")
                    nc.vector.tensor_tensor(out=x2, in0=xb16, in1=xb16,
                                            op=OP.mult)
                    mu_ps = po_psum.tile([1, HALF], F32, tag="p",
                                         name=f"lnmu{b}")
                    for ifc in range(4):
                        nc.tensor.matmul(mu_ps, invd[:, 0:1], xb16[:, ifc, :],
                                         start=(ifc == 0), stop=(ifc == 3))
                    mu = small.tile([1, HALF], F32, tag="mu", bufs=2, name="mu")
                    nc.vector.tensor_copy(mu, mu_ps)
                    m2_ps = po_psum.tile([1, HALF], F32, tag="p",
                                         name=f"lnm2{b}")
                    for ifc in range(4):
                        nc.tensor.matmul(m2_ps, invd[:, 1:2], x2[:, ifc, :],
                                         start=(ifc == 0), stop=(ifc == 3))
                    h["mu"], h["m2_ps"] = mu, m2_ps

                def t_var():
                    mu, m2_ps = h["mu"], h["m2_ps"]
                    m2 = small.tile([1, HALF], F32, tag="m2", bufs=2, name="m2")
                    nc.vector.tensor_tensor(out=m2, in0=mu, in1=mu, op=OP.mult)
                    ex2 = small.tile([1, HALF], F32, tag="ex2", bufs=2,
                                     name="ex2")
                    nc.vector.tensor_tensor(out=ex2, in0=m2_ps, in1=m2,
                                            op=OP.subtract)
                    nc.vector.tensor_scalar(out=ex2, in0=ex2, scalar1=LN_EPS,
                                            scalar2=None, op0=OP.add)
                    sd = small.tile([1, HALF], F32, tag="sd", bufs=2, name="sd")
                    sdi = sd.bitcast(I32)
                    nc.vector.tensor_scalar(out=sdi, in0=ex2.bitcast(I32),
                                            scalar1=1, scalar2=None,
                                            op0=OP.logical_shift_right)
                    nc.vector.tensor_tensor(out=sdi, in0=magic, in1=sdi,
                                            op=OP.subtract)
                    for _ in range(2):
                        nc.vector.tensor_tensor(out=m2, in0=ex2, in1=sd,
                                                op=OP.mult)
                        nc.vector.tensor_tensor(out=m2, in0=m2, in1=sd,
                                                op=OP.mult)
                        nc.vector.tensor_scalar(out=m2, in0=m2, scalar1=-0.5,
                                                scalar2=1.5, op0=OP.mult,
                                                op1=OP.add)
                        nc.vector.tensor_tensor(out=sd, in0=sd, in1=m2,
                                                op=OP.mult)
                    nc.vector.tensor_tensor(out=mu, in0=mu, in1=sd, op=OP.mult)
                    srep = po_psum.tile([128, 2, HALF], F32, tag="p",
                                        name=f"srep{b}")
                    nc.tensor.matmul(srep[:, 0, :], onesf[0:1, :], sd,
                                     start=True, stop=True)
                    nc.tensor.matmul(srep[:, 1, :], onesf[0:1, :], mu,
                                     start=True, stop=True)
                    h["srep"] = srep

                def t_apply():
                    x_b, srep = h["x"], h["srep"]
                    outb = act.tile([128, 4, HALF], BF16, tag=f"lnb{b}",
                                    bufs=2, name=f"lnb{b}")
                    outf = None
                    if resid is not None:
                        outf = act.tile([128, 4, HALF], F32, tag=f"lnf{b}",
                                        bufs=2, name=f"lnf{b}")
                    for ifc in range(4):
                        t1 = small.tile([128, HALF], F32, tag=f"t1{b}", bufs=2,
                                        name=f"t1{b}")
                        nc.vector.tensor_tensor(out=t1, in0=x_b[:, ifc, :],
                                                in1=srep[:, 0, :], op=OP.mult)
                        nc.vector.tensor_tensor(out=t1, in0=t1,
                                                in1=srep[:, 1, :],
                                                op=OP.subtract)
                        nc.vector.tensor_scalar(
                            out=outb[:, ifc, :], in0=t1,
                            scalar1=lng[:, ifc:ifc + 1],
                            scalar2=lnb[:, ifc:ifc + 1],
                            op0=OP.mult, op1=OP.add)
                        if outf is not None:
                            nc.vector.tensor_scalar(
                                out=outf[:, ifc, :], in0=t1,
                                scalar1=lng[:, ifc:ifc + 1],
                                scalar2=lnb[:, ifc:ifc + 1],
                                op0=OP.mult, op1=OP.add)
                    lnb_store[(m, b)] = outb
                    if resid is not None:
                        resid_f[b] = outf
                        final_f[b] = outf

                return [t_oproj, t_stats, t_var, t_apply]

            def qkv_exchange(b, x_b16):
                qt_b = proj_T_half(x_b16, wq, bq, f"qt{b}", p_psum)
                kt_b = proj_T_half(x_b16, wk, bk, f"kt{b}", p_psum)
                v_b = proj_V_half(x_b16, f"v{b}")
                return exchange_half(b, qt_b, kt_b, v_b)

            # Flat (layer, half) software pipeline.  Each attention phase
            # consumes, as interleaved fillers: (1) the previous half's
            # full tail (O-proj + LayerNorm), then (2) the next pending
            # (layer, half)'s QKV+pack+A2A.
            NLAYERS = 2 * NITER
            resid_f = list(x0h)            # f32 residual stream (per half)
            pk = {}
            pk[(0, 0)] = qkv_exchange(0, x0b[0])
            pk[(0, 1)] = qkv_exchange(1, x0b[1])
            lnb_store = {}
            final_f = [None, None]
            pending_tail = []
            for m in range(NLAYERS):
                for b in range(2):
                    tgt = (m, 1) if b == 0 else (m + 1, 0)
                    fillers = list(pending_tail)
                    pending_tail = []
                    if tgt not in pk and tgt[0] < NLAYERS:
                        src_key = (tgt[0] - 1, tgt[1])
                        fillers += proj_fillers(
                            tgt[1], (lambda k=src_key: lnb_store[k]),
                            tgt, pk)
                    z_b = attention_half(b, *pk[(m, b)], fillers=fillers)
                    pending_tail = make_tail(
                        m, b, z_b,
                        resid_f[b] if m % 2 == 1 else None)
            for f in pending_tail:
                f()

            for b in range(2):
                nc.sync.dma_start(
                    out=out_d.rearrange("(c p) f -> p c f", p=128)[:, :, 256 * b:256 * (b + 1)],
                    in_=final_f[b])
    return nc


_NC_CACHE = None


def _get_nc():
    global _NC_CACHE
    if _NC_CACHE is None:
        nc = bacc.Bacc("TRN2", target_bir_lowering=False, debug=False,
                       num_devices=NCORES)
        _build_graph(nc)
        nc.compile()
        _NC_CACHE = nc
    return _NC_CACHE


def kernel(encoder_inputs, Wq, bq, Wk, bk, Wv, bv, Wo, bo, ln_g, ln_b,
           _trace=False, _trace_kwargs=None):
    BF = ml_dtypes.bfloat16
    x = np.asarray(encoder_inputs, dtype=np.float32)
    consts = {
        "wq": np.ascontiguousarray(np.asarray(Wq, np.float32).astype(BF)),
        "wk": np.ascontiguousarray(np.asarray(Wk, np.float32).astype(BF)),
        "wv": np.ascontiguousarray(np.asarray(Wv, np.float32).astype(BF)),
        "wo": np.ascontiguousarray(np.asarray(Wo, np.float32).astype(BF)),
        "bq": np.ascontiguousarray(np.asarray(bq, np.float32).reshape(4, 128).T),
        "bk": np.ascontiguousarray(np.asarray(bk, np.float32).reshape(4, 128).T),
        "bo": np.ascontiguousarray(np.asarray(bo, np.float32).reshape(4, 128).T),
        "bv": np.asarray(bv, np.float32).astype(BF).reshape(1, DIM),
        "lng": np.ascontiguousarray(np.asarray(ln_g, np.float32).reshape(4, 128).T),
        "lnb": np.ascontiguousarray(np.asarray(ln_b, np.float32).reshape(4, 128).T),
        "ones": np.ones((128, 128), BF),
        "invd": np.full((128, 2), 1.0 / DIM, BF),
    }
    in_maps = []
    for c in range(NCORES):
        xt = np.concatenate([x[0, 256 * c:256 * (c + 1)].T,
                             x[1, 256 * c:256 * (c + 1)].T], axis=1)
        in_maps.append({"xt": np.ascontiguousarray(xt), **consts})

    nc = _get_nc()
    res = bass_utils.run_bass_kernel_spmd(
        nc, in_maps, core_ids=list(range(NCORES)),
        trace=_trace, **(_trace_kwargs or {}))

    out = np.zeros((2, 2048, DIM), np.float32)
    for c in range(NCORES):
        r = res.results[c]["out"]
        out[0, 256 * c:256 * (c + 1)] = r[:, :256].T
        out[1, 256 * c:256 * (c + 1)] = r[:, 256:].T
    if _trace:
        kernel._last_results = res
    return out


# revision 23
# speedup vs baseline: 1.0164x; 1.0001x over previous
"""Distributed Trainium2 kernel for nn_Encoder_88502096101469.

8-core SPMD layout (one NEFF, per-core data):
- Activations live TRANSPOSED in SBUF as batch-halves: X^T_b (512 feat x
  256 cols) where cols = batch-b rows [256c, 256c+256) for core c.
- Core c owns attention head h=c for BOTH batches. The torch-faithful
  "raw reshape" of (b, h, t, dv) -> (b, t, h*dv) maps head h's output to
  Z rows [256h, 256h+256) per batch, which is exactly core c's resident
  row range -> no post-attention exchange needed.
- Per batch, one 8-way AllToAll exchanges Q^T/K^T slices (64 head rows x
  256 local cols, bf16) and V natural slices (256 rows x 64 head cols).
- The whole layer is pipelined by batch-half: projections, pack, A2A,
  O-projection and LayerNorm of one half overlap attention of the other.
- v3 vs baseline:
  * All matmuls bf16 (projections were fp32 HIGH mode = 2 cyc/col with
    3x slower LDWEIGHTS).
  * Score matmuls run as concurrent row-tiled pairs (K=64 contraction
    only fills half the PE rows; two key-chunks execute simultaneously
    on row groups 0-1 / 2-3 via qh/kh replicated to partitions 64-127).
  * Softmax exp split between ScalarE (table exp) and VectorE
    (Schraudolph bf16 bit-trick: bits16 = int16(A*s + B)) so the
    2048x2048 exp stream is not serialized on one engine.
  * PSUM tags decoupled: next-layer Q/K/V projections never share a
    psum buffer with O-proj/LN of the current layer, so the QKV->pack->
    AllToAll chain for layer l+1 overlaps attention of layer l.
  * attV accumulator evacuated to SBUF immediately (frees the single
    o-psum bank; recip/normalize run off the SBUF copy).
  * A2A pack/unpack DMAs consolidated (9 big DMAs instead of 29).
  * The residual / LayerNorm stream stays in f32 (bf16 storage there
    compounds to ~5e-2 max error over 6 layers); projections read a
    bf16 copy made once per LN.
- Softmax skips max-subtraction (logits >= 0, O(1)); the denominator
  comes from a ones-column appended to V (lhsT M=65); exp folds the 1/8.
"""
import numpy as np
import ml_dtypes

import concourse.bass as bass
import concourse.bacc as bacc
import concourse.tile as tile
from concourse import mybir
from concourse import bass_utils

NCORES = 8
DIM = 512
HALF = 256          # per-core cols per batch
NITER = 3           # LAYERS + 1
LN_EPS = 1e-5

F32 = mybir.dt.float32
BF16 = mybir.dt.bfloat16
I16 = mybir.dt.int16
FP8 = mybir.dt.float8e4
I32 = mybir.dt.int32
AF = mybir.ActivationFunctionType
OP = mybir.AluOpType

# Schraudolph bf16 exp: bits16 = int16(A*s_raw + B) where s_raw = q.k
# (the 1/8 logit scale is folded into A).  ~3% max relative error; the
# softmax ratio and 2048-key averaging wash it out (validated in numpy
# and on hardware: 3.4% elementwise, end-to-end insensitive).
EXP_A = (128.0 / np.log(2.0)) * 0.125
EXP_B = 16256.0 - 128.0 * 0.0465

# A2A per-batch shard layout (flat bf16 words per (src,dst) pair):
#   [0:16384)      Q^T slice  (64 of-rows, 256 cols)
#   [16384:32768)  K^T slice  (64 of-rows, 256 cols)
#   [32768:49152)  V slice    (2 t-chunks, 128 rows, 64 fv-cols)
SHARD = 49152


def _build_graph(nc):
    xt_in = nc.dram_tensor("xt", [DIM, 2 * HALF], F32, kind="ExternalInput").ap()
    wq_in = nc.dram_tensor("wq", [DIM, DIM], BF16, kind="ExternalInput").ap()
    wk_in = nc.dram_tensor("wk", [DIM, DIM], BF16, kind="ExternalInput").ap()
    wv_in = nc.dram_tensor("wv", [DIM, DIM], BF16, kind="ExternalInput").ap()
    wo_in = nc.dram_tensor("wo", [DIM, DIM], BF16, kind="ExternalInput").ap()
    bq_in = nc.dram_tensor("bq", [128, 4], F32, kind="ExternalInput").ap()
    bk_in = nc.dram_tensor("bk", [128, 4], F32, kind="ExternalInput").ap()
    bo_in = nc.dram_tensor("bo", [128, 4], F32, kind="ExternalInput").ap()
    bv_in = nc.dram_tensor("bv", [1, DIM], BF16, kind="ExternalInput").ap()
    lng_in = nc.dram_tensor("lng", [128, 4], F32, kind="ExternalInput").ap()
    lnb_in = nc.dram_tensor("lnb", [128, 4], F32, kind="ExternalInput").ap()
    ones_in = nc.dram_tensor("ones", [128, 128], BF16, kind="ExternalInput").ap()
    invd_in = nc.dram_tensor("invd", [128, 2], BF16, kind="ExternalInput").ap()
    out_d = nc.dram_tensor("out", [DIM, 2 * HALF], F32, kind="ExternalOutput").ap()

    groups = [list(range(NCORES))]

    from contextlib import ExitStack
    with tile.TileContext(nc) as tc, ExitStack() as ctx:
        const = ctx.enter_context(tc.tile_pool(name="const", bufs=1))
        act = ctx.enter_context(tc.tile_pool(name="act", bufs=1))
        qkv = ctx.enter_context(tc.tile_pool(name="qkv", bufs=1))
        gath = ctx.enter_context(tc.tile_pool(name="gath", bufs=2))
        epool = ctx.enter_context(tc.tile_pool(name="epool", bufs=3))
        small = ctx.enter_context(tc.tile_pool(name="small", bufs=1))
        dram = ctx.enter_context(tc.tile_pool(name="dram", bufs=1, space="DRAM"))
        s_psum = ctx.enter_context(tc.tile_pool(name="s_psum", bufs=2, space="PSUM"))
        o_psum = ctx.enter_context(tc.tile_pool(name="o_psum", bufs=1, space="PSUM"))
        p_psum = ctx.enter_context(tc.tile_pool(name="p_psum", bufs=2, space="PSUM"))
        po_psum = ctx.enter_context(tc.tile_pool(name="po_psum", bufs=1, space="PSUM"))
        if True:
            # ---- constants to SBUF ----
            def load_w(ap_in, nm):
                t = const.tile([128, 4, DIM], BF16, name=nm, tag=nm)
                nc.sync.dma_start(out=t, in_=ap_in.rearrange("(c p) f -> p c f", p=128))
                return t

            wq, wk, wv, wo = (load_w(wq_in, "wqt"), load_w(wk_in, "wkt"),
                              load_w(wv_in, "wvt"), load_w(wo_in, "wot"))
            bq = const.tile([128, 4], F32)
            bk = const.tile([128, 4], F32)
            bo = const.tile([128, 4], F32)
            lng = const.tile([128, 4], F32)
            lnb = const.tile([128, 4], F32)
            for t, a in ((bq, bq_in), (bk, bk_in), (bo, bo_in), (lng, lng_in), (lnb, lnb_in)):
                nc.sync.dma_start(out=t, in_=a)
            bv = const.tile([1, DIM], BF16)
            nc.sync.dma_start(out=bv, in_=bv_in)
            ones = const.tile([128, 128], BF16)
            nc.sync.dma_start(out=ones, in_=ones_in)
            invd = const.tile([128, 2], BF16)
            nc.sync.dma_start(out=invd, in_=invd_in)
            magic = const.tile([1, HALF], I32)
            nc.vector.memset(magic, 0x5F3759DF)
            onesf = const.tile([1, 128], F32)
            nc.vector.memset(onesf, 1.0)

            # initial activation, as halves: f32 residual + bf16 proj copy
            x0h, x0b = [], []
            for b in range(2):
                xb = act.tile([128, 4, HALF], F32, tag=f"x0h{b}", name=f"x0h{b}")
                nc.sync.dma_start(
                    out=xb,
                    in_=xt_in.rearrange("(c p) f -> p c f", p=128)[:, :, 256 * b:256 * (b + 1)])
                x0h.append(xb)
                xbb = act.tile([128, 4, HALF], BF16, tag=f"x0b{b}", name=f"x0b{b}")
                nc.vector.tensor_copy(xbb, xb)
                x0b.append(xbb)

            # DRAM bounce buffers
            sendb = [dram.tile([NCORES, SHARD], BF16, tag=f"send{b}",
                               name=f"send{b}") for b in range(2)]
            recvb = [dram.tile([NCORES, SHARD], BF16, tag=f"recv{b}",
                               name=f"recv{b}") for b in range(2)]
            rs_d = dram.tile([1, 512], F32, tag="rs_d", name="rs_d", bufs=2)
            stat_d = [dram.tile([2, HALF], F32, tag=f"stat{b}",
                                name=f"stat{b}", bufs=2) for b in range(2)]

            def proj_T_half(x_b, w, btile, tag, pool, odt=BF16):
                """(128,4,HALF) <- relu(w^T x_b + bias), transposed output."""
                out = qkv.tile([128, 4, HALF], odt, tag=tag, name=tag)
                for pair in range(2):
                    ps = pool.tile([128, 2, HALF], F32, tag="p", name=f"ps_{tag}")
                    for i in range(2):
                        ofc = 2 * pair + i
                        for ifc in range(4):
                            nc.tensor.matmul(
                                ps[:, i, :],
                                w[:, ifc, 128 * ofc:128 * (ofc + 1)],
                                x_b[:, ifc, :],
                                start=(ifc == 0), stop=(ifc == 3))
                        nc.vector.tensor_scalar(
                            out=out[:, ofc, :], in0=ps[:, i, :],
                            scalar1=btile[:, ofc:ofc + 1], scalar2=0.0,
                            op0=OP.add, op1=OP.max)
                return out

            def proj_V_half(x_b, tag):
                """(128,2,DIM) bf16 <- relu(x_b^T wv + bv), natural layout."""
                out = qkv.tile([128, 2, DIM], BF16, tag=tag, name=tag)
                for tch in range(2):
                    ps = p_psum.tile([128, DIM], F32, tag="p", name=f"ps_{tag}{tch}")
                    for ifc in range(4):
                        nc.tensor.matmul(
                            ps,
                            x_b[:, ifc, 128 * tch:128 * (tch + 1)],
                            wv[:, ifc, :],
                            start=(ifc == 0), stop=False)
                    nc.tensor.matmul(
                        ps, ones[0:1, :], bv, start=False, stop=True)
                    nc.vector.tensor_scalar(
                        out=out[:, tch, :], in0=ps,
                        scalar1=0.0, scalar2=None, op0=OP.max)
                return out

            def exchange_half(b, qt_b, kt_b, v_b):
                sb, rb = sendb[b], recvb[b]
                # pack Q, K: 2 DMAs each (one per 64-partition half)
                for base, src in ((0, qt_b), (16384, kt_b)):
                    seg = sb[:, base:base + 16384].rearrange(
                        "(cq two) (r c) -> two r cq c", two=2, c=256)
                    for two in range(2):
                        nc.sync.dma_start(
                            out=seg[two],
                            in_=src[64 * two:64 * (two + 1), :, :])
                # pack V: 2 DMAs (one per t-chunk; 4-dim APs don't balance)
                for tc in range(2):
                    nc.sync.dma_start(
                        out=sb[:, 32768 + 8192 * tc:32768 + 8192 * (tc + 1)]
                            .rearrange("d (p j) -> p d j", p=128),
                        in_=v_b[:, tc, :].rearrange("p (d j) -> p d j", j=64))
                nc.gpsimd.collective_compute(
                    "AllToAll", OP.bypass, replica_groups=groups,
                    ins=[sb.opt()], outs=[rb.opt()])
                # unpack with 64->128 replication for row-tiled score matmuls
                qh = gath.tile([128, 8, 256], BF16, tag=f"qh{b}", name=f"qh{b}")
                kh = gath.tile([128, 8, 256], BF16, tag=f"kh{b}", name=f"kh{b}")
                for dst, base in ((qh, 0), (kh, 16384)):
                    for hh in range(2):
                        nc.sync.dma_start(
                            out=dst[64 * hh:64 * (hh + 1), :, :],
                            in_=rb[:, base:base + 16384]
                                .rearrange("s (r c) -> r s c", r=64))
                vh = gath.tile([128, 16, 65], BF16, tag=f"vh{b}", name=f"vh{b}")
                for tc2 in range(2):
                    nc.sync.dma_start(
                        out=vh[:, tc2::2, 0:64],
                        in_=rb[:, 32768 + 8192 * tc2:32768 + 8192 * (tc2 + 1)]
                            .rearrange("s (p j) -> p s j", p=128))
                nc.vector.memset(vh[:, :, 64:65], 1.0)
                # fp8 copy (Ko-step padded to 80 for DoubleRow alignment)
                vh8 = gath.tile([128, 16, 80], FP8, tag=f"vh8{b}", name=f"vh8{b}")
                nc.vector.tensor_copy(vh8[:, :, 0:65], vh)
                return qh, kh, vh, vh8

            def attention_half(b, qh, kh, vh, vh8, fillers=()):
                """(128,4,HALF) bf16 Z^T for batch b (local Z rows).

                `fillers` are thunks (next-layer projection chunks) called
                one per score group so their PE matmuls land BETWEEN
                attention matmuls in the engine FIFOs -- the only way to
                fill the PE during the exp-bound attention stream.
                The per-column normalize (recip + z-scale) is deferred to
                the end so its DMA bounce never head-of-line-blocks the
                DVE queue between exp groups.
                """
                fill_iter = iter(fillers)
                z = qkv.tile([128, 4, HALF], BF16, tag=f"z{b}", name=f"z{b}")
                cols = []
                for j in range(4):
                    ops = o_psum.tile([65, 512], F32, tag="o", name=f"ops{b}{j}")
                    dve_groups = (2, 5) if j % 2 == 0 else (4,)
                    for g in range(8):
                        sps = s_psum.tile([128, 2, 512], F32, tag="s",
                                          name=f"sps{b}{j}{g}")
                        for u in range(2):
                            k = 2 * g + u
                            nc.tensor.matmul(
                                sps[:, u, :],
                                kh[64 * u:64 * (u + 1), k // 2,
                                   128 * (k % 2):128 * (k % 2 + 1)],
                                qh[64 * u:64 * (u + 1), 2 * j:2 * j + 2, :],
                                start=True, stop=True)
                        if g in dve_groups:
                            e = epool.tile([128, 2, 512], BF16, tag="e",
                                           name=f"e{b}{j}{g}")
                            nc.vector.tensor_scalar(
                                out=e.bitcast(I16), in0=sps,
                                scalar1=float(EXP_A), scalar2=float(EXP_B),
                                op0=OP.mult, op1=OP.add)
                            for u in range(2):
                                k = 2 * g + u
                                nc.tensor.matmul(
                                    ops, vh[:, k, :], e[:, u, :],
                                    start=False, stop=False)
                        else:
                            e8 = epool.tile([128, 2, 512], FP8, tag="e",
                                            name=f"e{b}{j}{g}")
                            nc.scalar.activation(e8, sps, AF.Exp, scale=0.125)
                            nc.tensor.matmul(
                                ops, vh8[:, 2 * g:2 * g + 2, 0:65], e8,
                                start=(g == 0), stop=(g == 7),
                                perf_mode=mybir.MatmulPerfMode.DoubleRow)
                        f = next(fill_iter, None)
                        if f is not None:
                            f()
                    # evacuate the single o-psum bank immediately; bounce
                    # the denominator row out for the 64-way broadcast
                    oc = small.tile([65, 512], F32, tag="oc", bufs=4, name="oc")
                    nc.vector.tensor_copy(oc, ops)
                    nc.sync.dma_start(out=rs_d, in_=oc[64:65, :])
                    dvec = small.tile([64, 512], F32, tag="dvec", bufs=4, name="dvec")
                    nc.sync.dma_start(
                        out=dvec, in_=rs_d.partition_broadcast(64)[:, 0, :])
                    cols.append((oc, dvec))
                for f in fill_iter:
                    f()
                for j, (oc, dvec) in enumerate(cols):
                    rrep = small.tile([64, 512], F32, tag="rrep", bufs=2, name="rrep")
                    nc.vector.reciprocal_approx_fast(rrep, dvec)
                    o_v = oc[0:64, :].rearrange("f (r s) -> f s r", s=8)
                    r_v = rrep.rearrange("f (r s) -> f s r", s=8)
                    for q in range(2):
                        nc.vector.tensor_tensor(
                            out=z[64 * q:64 * (q + 1), :, 64 * j:64 * (j + 1)],
                            in0=o_v[:, q::2, :],
                            in1=r_v[:, q::2, :],
                            op=OP.mult)
                return z

            def proj_fillers(b, get_x, key, pk_store):
                """Thunks that together emit QKV proj + pack + A2A + unpack
                for (layer, half) = key, sliced so one thunk fits in one
                attention score-group's PE shadow."""
                tiles = {}

                def out_tile(tag, shape):
                    if tag not in tiles:
                        tiles[tag] = qkv.tile(shape, BF16, tag=tag, name=tag)
                    return tiles[tag]

                def t_pair(w, btile, tag, pair):
                    def run():
                        x_b16 = get_x()
                        out = out_tile(tag, [128, 4, HALF])
                        ps = p_psum.tile([128, 2, HALF], F32, tag="p",
                                         name=f"ps_{tag}{pair}")
                        for i in range(2):
                            ofc = 2 * pair + i
                            for ifc in range(4):
                                nc.tensor.matmul(
                                    ps[:, i, :],
                                    w[:, ifc, 128 * ofc:128 * (ofc + 1)],
                                    x_b16[:, ifc, :],
                                    start=(ifc == 0), stop=(ifc == 3))
                            nc.vector.tensor_scalar(
                                out=out[:, ofc, :], in0=ps[:, i, :],
                                scalar1=btile[:, ofc:ofc + 1], scalar2=0.0,
                                op0=OP.add, op1=OP.max)
                    return run

                def v_chunk(tch):
                    def run():
                        x_b16 = get_x()
                        out = out_tile(f"v{b}", [128, 2, DIM])
                        ps = p_psum.tile([128, DIM], F32, tag="p",
                                         name=f"ps_v{b}{tch}")
                        for ifc in range(4):
                            nc.tensor.matmul(
                                ps,
                                x_b16[:, ifc, 128 * tch:128 * (tch + 1)],
                                wv[:, ifc, :],
                                start=(ifc == 0), stop=False)
                        nc.tensor.matmul(
                            ps, ones[0:1, :], bv, start=False, stop=True)
                        nc.vector.tensor_scalar(
                            out=out[:, tch, :], in0=ps,
                            scalar1=0.0, scalar2=None, op0=OP.max)
                    return run

                def xchg():
                    pk_store[key] = exchange_half(
                        b, tiles[f"qt{b}"], tiles[f"kt{b}"], tiles[f"v{b}"])

                return [t_pair(wq, bq, f"qt{b}", 0), t_pair(wq, bq, f"qt{b}", 1),
                        t_pair(wk, bk, f"kt{b}", 0), t_pair(wk, bk, f"kt{b}", 1),
                        v_chunk(0), v_chunk(1), xchg]

            def make_tail(m, b, z_b, resid):
                """Thunks for the post-attention tail of (m, b): O-proj,
                LN stats, variance/rsqrt chain, LN apply.  Consumed as
                fillers inside the NEXT attention phase so these PE ops
                (behind the serial LN DVE chain) never head-of-line-block
                the next attention stream in the engine FIFOs."""
                h = {}

                def t_oproj():
                    h["y"] = proj_T_half(z_b, wo, bo, f"y{b}", po_psum,
                                         odt=F32)

                def t_stats():
                    x_b = h["y"]
                    if resid is not None:
                        xr = act.tile([128, 4, HALF], F32, tag=f"xr{b}",
                                      name=f"xr{b}")
                        nc.vector.tensor_tensor(out=xr, in0=x_b, in1=resid,
                                                op=OP.add)
                        x_b = xr
                    h["x"] = x_b
                    xb16 = act.tile([128, 4, HALF], BF16, tag=f"xb{b}",
                                    name=f"xb{b}")
                    nc.vector.tensor_copy(xb16, x_b)
                    x2 = act.tile([128, 4, HALF], BF16, tag=f"x2{b}",
                                  name=f"x2{b}")
                    nc.vector.tensor_tensor(out=x2, in0=xb16, in1=xb16,
                                            op=OP.mult)
                    mu_ps = po_psum.tile([1, HALF], F32, tag="p",
                                         name=f"lnmu{b}")
                    for ifc in range(4):
                        nc.tensor.matmul(mu_ps, invd[:, 0:1], xb16[:, ifc, :],
                                         start=(ifc == 0), stop=(ifc == 3))
                    mu = small.tile([1, HALF], F32, tag="mu", bufs=2, name="mu")
                    nc.vector.tensor_copy(mu, mu_ps)
                    m2_ps = po_psum.tile([1, HALF], F32, tag="p",
                                         name=f"lnm2{b}")
                    for ifc in range(4):
                        nc.tensor.matmul(m2_ps, invd[:, 1:2], x2[:, ifc, :],
                                         start=(ifc == 0), stop=(ifc == 3))
                    h["mu"], h["m2_ps"] = mu, m2_ps

                def t_var():
                    mu, m2_ps = h["mu"], h["m2_ps"]
                    m2 = small.tile([1, HALF], F32, tag="m2", bufs=2, name="m2")
                    nc.vector.tensor_tensor(out=m2, in0=mu, in1=mu, op=OP.mult)
                    ex2 = small.tile([1, HALF], F32, tag="ex2", bufs=2,
                                     name="ex2")
                    nc.vector.tensor_tensor(out=ex2, in0=m2_ps, in1=m2,
                                            op=OP.subtract)
                    nc.vector.tensor_scalar(out=ex2, in0=ex2, scalar1=LN_EPS,
                                            scalar2=None, op0=OP.add)
                    sd = small.tile([1, HALF], F32, tag="sd", bufs=2, name="sd")
                    sdi = sd.bitcast(I32)
                    nc.vector.tensor_scalar(out=sdi, in0=ex2.bitcast(I32),
                                            scalar1=1, scalar2=None,
                                            op0=OP.logical_shift_right)
                    nc.vector.tensor_tensor(out=sdi, in0=magic, in1=sdi,
                                            op=OP.subtract)
                    for _ in range(2):
                        nc.vector.tensor_tensor(out=m2, in0=ex2, in1=sd,
                                                op=OP.mult)
                        nc.vector.tensor_tensor(out=m2, in0=m2, in1=sd,
                                                op=OP.mult)
                        nc.vector.tensor_scalar(out=m2, in0=m2, scalar1=-0.5,
                                                scalar2=1.5, op0=OP.mult,
                                                op1=OP.add)
                        nc.vector.tensor_tensor(out=sd, in0=sd, in1=m2,
                                                op=OP.mult)
                    nc.vector.tensor_tensor(out=mu, in0=mu, in1=sd, op=OP.mult)
                    srep = po_psum.tile([128, 2, HALF], F32, tag="p",
                                        name=f"srep{b}")
                    nc.tensor.matmul(srep[:, 0, :], onesf[0:1, :], sd,
                                     start=True, stop=True)
                    nc.tensor.matmul(srep[:, 1, :], onesf[0:1, :], mu,
                                     start=True, stop=True)
                    h["srep"] = srep

                def t_apply():
                    x_b, srep = h["x"], h["srep"]
                    outb = act.tile([128, 4, HALF], BF16, tag=f"lnb{b}",
                                    bufs=2, name=f"lnb{b}")
                    outf = None
                    if resid is not None:
                        outf = act.tile([128, 4, HALF], F32, tag=f"lnf{b}",
                                        bufs=2, name=f"lnf{b}")
                    for ifc in range(4):
                        t1 = small.tile([128, HALF], F32, tag=f"t1{b}", bufs=2,
                                        name=f"t1{b}")
                        nc.vector.tensor_tensor(out=t1, in0=x_b[:, ifc, :],
                                                in1=srep[:, 0, :], op=OP.mult)
                        nc.vector.tensor_tensor(out=t1, in0=t1,
                                                in1=srep[:, 1, :],
                                                op=OP.subtract)
                        nc.vector.tensor_scalar(
                            out=outb[:, ifc, :], in0=t1,
                            scalar1=lng[:, ifc:ifc + 1],
                            scalar2=lnb[:, ifc:ifc + 1],
                            op0=OP.mult, op1=OP.add)
                        if outf is not None:
                            nc.vector.tensor_scalar(
                                out=outf[:, ifc, :], in0=t1,
                                scalar1=lng[:, ifc:ifc + 1],
                                scalar2=lnb[:, ifc:ifc + 1],
                                op0=OP.mult, op1=OP.add)
                    lnb_store[(m, b)] = outb
                    if resid is not None:
                        resid_f[b] = outf
                        final_f[b] = outf

                return [t_oproj, t_stats, t_var, t_apply]

            def qkv_exchange(b, x_b16):
                qt_b = proj_T_half(x_b16, wq, bq, f"qt{b}", p_psum)
                kt_b = proj_T_half(x_b16, wk, bk, f"kt{b}", p_psum)
                v_b = proj_V_half(x_b16, f"v{b}")
                return exchange_half(b, qt_b, kt_b, v_b)

            # Flat (layer, half) software pipeline.  Each attention phase
            # consumes, as interleaved fillers: (1) the previous half's
            # full tail (O-proj + LayerNorm), then (2) the next pending
            # (layer, half)'s QKV+pack+A2A.
            NLAYERS = 2 * NITER
            resid_f = list(x0h)            # f32 residual stream (per half)
            pk = {}
            pk[(0, 0)] = qkv_exchange(0, x0b[0])
            pk[(0, 1)] = qkv_exchange(1, x0b[1])
            lnb_store = {}
            final_f = [None, None]
            pending_tail = []
            for m in range(NLAYERS):
                for b in range(2):
                    tgt = (m, 1) if b == 0 else (m + 1, 0)
                    fillers = list(pending_tail)
                    pending_tail = []
                    if tgt not in pk and tgt[0] < NLAYERS:
                        src_key = (tgt[0] - 1, tgt[1])
                        fillers += proj_fillers(
                            tgt[1], (lambda k=src_key: lnb_store[k]),
                            tgt, pk)
                    z_b = attention_half(b, *pk[(m, b)], fillers=fillers)
                    pending_tail = make_tail(
                        m, b, z_b,
                        resid_f[b] if m % 2 == 1 else None)
            for f in pending_tail:
                f()

            for b in range(2):
                nc.sync.dma_start(
                    out=out_d.rearrange("(c p) f -> p c f", p=128)[:, :, 256 * b:256 * (b + 1)],
                    in_=final_f[b])
    return nc


_NC_CACHE = None


def _get_nc():
    global _NC_CACHE
    if _NC_CACHE is None:
        nc = bacc.Bacc("TRN2", target_bir_lowering=False, debug=False,
                       num_devices=NCORES)
        _build_graph(nc)
        nc.compile()
        _NC_CACHE = nc
    return _NC_CACHE


def kernel(encoder_inputs, Wq, bq, Wk, bk, Wv, bv, Wo, bo, ln_g, ln_b,
           _trace=False, _trace_kwargs=None):
    BF = ml_dtypes.bfloat16
    x = np.asarray(encoder_inputs, dtype=np.float32)
    consts = {
        "wq": np.ascontiguousarray(np.asarray(Wq, np.float32).astype(BF)),
        "wk": np.ascontiguousarray(np.asarray(Wk, np.float32).astype(BF)),
        "wv": np.ascontiguousarray(np.asarray(Wv, np.float32).astype(BF)),
        "wo": np.ascontiguousarray(np.asarray(Wo, np.float32).astype(BF)),
        "bq": np.ascontiguousarray(np.asarray(bq, np.float32).reshape(4, 128).T),
        "bk": np.ascontiguousarray(np.asarray(bk, np.float32).reshape(4, 128).T),
        "bo": np.ascontiguousarray(np.asarray(bo, np.float32).reshape(4, 128).T),
        "bv": np.asarray(bv, np.float32).astype(BF).reshape(1, DIM),
        "lng": np.ascontiguousarray(np.asarray(ln_g, np.float32).reshape(4, 128).T),
        "lnb": np.ascontiguousarray(np.asarray(ln_b, np.float32).reshape(4, 128).T),
        "ones": np.ones((128, 128), BF),
        "invd": np.full((128, 2), 1.0 / DIM, BF),
    }
    in_maps = []
    for c in range(NCORES):
        xt = np.concatenate([x[0, 256 * c:256 * (c + 1)].T,
                             x[1, 256 * c:256 * (c + 1)].T], axis=1)
        in_maps.append({"xt": np.ascontiguousarray(xt), **consts})

    nc = _get_nc()
    res = bass_utils.run_bass_kernel_spmd(
        nc, in_maps, core_ids=list(range(NCORES)),
        trace=_trace, **(_trace_kwargs or {}))

    out = np.zeros((2, 2048, DIM), np.float32)
    for c in range(NCORES):
        r = res.results[c]["out"]
        out[0, 256 * c:256 * (c + 1)] = r[:, :256].T
        out[1, 256 * c:256 * (c + 1)] = r[:, 256:].T
    if _trace:
        kernel._last_results = res
    return out


# revision 24
# speedup vs baseline: 1.0292x; 1.0126x over previous
"""Distributed Trainium2 kernel for nn_Encoder_88502096101469.

8-core SPMD layout (one NEFF, per-core data):
- Activations live TRANSPOSED in SBUF as batch-halves: X^T_b (512 feat x
  256 cols) where cols = batch-b rows [256c, 256c+256) for core c.
- Core c owns attention head h=c for BOTH batches. The torch-faithful
  "raw reshape" of (b, h, t, dv) -> (b, t, h*dv) maps head h's output to
  Z rows [256h, 256h+256) per batch, which is exactly core c's resident
  row range -> no post-attention exchange needed.
- Per batch, one 8-way AllToAll exchanges Q^T/K^T slices (64 head rows x
  256 local cols, bf16) and V natural slices (256 rows x 64 head cols).
- The whole layer is pipelined by batch-half: projections, pack, A2A,
  O-projection and LayerNorm of one half overlap attention of the other.
- v3 vs baseline:
  * All matmuls bf16 (projections were fp32 HIGH mode = 2 cyc/col with
    3x slower LDWEIGHTS).
  * Score matmuls run as concurrent row-tiled pairs (K=64 contraction
    only fills half the PE rows; two key-chunks execute simultaneously
    on row groups 0-1 / 2-3 via qh/kh replicated to partitions 64-127).
  * Softmax exp split between ScalarE (table exp) and VectorE
    (Schraudolph bf16 bit-trick: bits16 = int16(A*s + B)) so the
    2048x2048 exp stream is not serialized on one engine.
  * PSUM tags decoupled: next-layer Q/K/V projections never share a
    psum buffer with O-proj/LN of the current layer, so the QKV->pack->
    AllToAll chain for layer l+1 overlaps attention of layer l.
  * attV accumulator evacuated to SBUF immediately (frees the single
    o-psum bank; recip/normalize run off the SBUF copy).
  * A2A pack/unpack DMAs consolidated (9 big DMAs instead of 29).
  * The residual / LayerNorm stream stays in f32 (bf16 storage there
    compounds to ~5e-2 max error over 6 layers); projections read a
    bf16 copy made once per LN.
- Softmax skips max-subtraction (logits >= 0, O(1)); the denominator
  comes from a ones-column appended to V (lhsT M=65); exp folds the 1/8.
"""
import numpy as np
import ml_dtypes

import concourse.bass as bass
import concourse.bacc as bacc
import concourse.tile as tile
from concourse import mybir
from concourse import bass_utils

NCORES = 8
DIM = 512
HALF = 256          # per-core cols per batch
NITER = 3           # LAYERS + 1
LN_EPS = 1e-5

F32 = mybir.dt.float32
BF16 = mybir.dt.bfloat16
I16 = mybir.dt.int16
I32 = mybir.dt.int32
AF = mybir.ActivationFunctionType
OP = mybir.AluOpType

# Schraudolph bf16 exp: bits16 = int16(A*s_raw + B) where s_raw = q.k
# (the 1/8 logit scale is folded into A).  ~3% max relative error; the
# softmax ratio and 2048-key averaging wash it out (validated in numpy
# and on hardware: 3.4% elementwise, end-to-end insensitive).
EXP_A = (128.0 / np.log(2.0)) * 0.125
EXP_B = 16256.0 - 128.0 * 0.0465

# A2A per-batch shard layout (flat bf16 words per (src,dst) pair):
#   [0:16384)      Q^T slice  (64 of-rows, 256 cols)
#   [16384:32768)  K^T slice  (64 of-rows, 256 cols)
#   [32768:49152)  V slice    (2 t-chunks, 128 rows, 64 fv-cols)
SHARD = 49152


def _build_graph(nc):
    xt_in = nc.dram_tensor("xt", [DIM, 2 * HALF], F32, kind="ExternalInput").ap()
    wq_in = nc.dram_tensor("wq", [DIM, DIM], BF16, kind="ExternalInput").ap()
    wk_in = nc.dram_tensor("wk", [DIM, DIM], BF16, kind="ExternalInput").ap()
    wv_in = nc.dram_tensor("wv", [DIM, DIM], BF16, kind="ExternalInput").ap()
    wo_in = nc.dram_tensor("wo", [DIM, DIM], BF16, kind="ExternalInput").ap()
    bq_in = nc.dram_tensor("bq", [128, 4], F32, kind="ExternalInput").ap()
    bk_in = nc.dram_tensor("bk", [128, 4], F32, kind="ExternalInput").ap()
    bo_in = nc.dram_tensor("bo", [128, 4], F32, kind="ExternalInput").ap()
    bv_in = nc.dram_tensor("bv", [1, DIM], BF16, kind="ExternalInput").ap()
    lng_in = nc.dram_tensor("lng", [128, 4], F32, kind="ExternalInput").ap()
    lnb_in = nc.dram_tensor("lnb", [128, 4], F32, kind="ExternalInput").ap()
    ones_in = nc.dram_tensor("ones", [128, 128], BF16, kind="ExternalInput").ap()
    invd_in = nc.dram_tensor("invd", [128, 2], BF16, kind="ExternalInput").ap()
    out_d = nc.dram_tensor("out", [DIM, 2 * HALF], F32, kind="ExternalOutput").ap()

    groups = [list(range(NCORES))]

    from contextlib import ExitStack
    with tile.TileContext(nc) as tc, ExitStack() as ctx:
        const = ctx.enter_context(tc.tile_pool(name="const", bufs=1))
        act = ctx.enter_context(tc.tile_pool(name="act", bufs=1))
        qkv = ctx.enter_context(tc.tile_pool(name="qkv", bufs=1))
        gath = ctx.enter_context(tc.tile_pool(name="gath", bufs=2))
        epool = ctx.enter_context(tc.tile_pool(name="epool", bufs=3))
        small = ctx.enter_context(tc.tile_pool(name="small", bufs=1))
        dram = ctx.enter_context(tc.tile_pool(name="dram", bufs=1, space="DRAM"))
        s_psum = ctx.enter_context(tc.tile_pool(name="s_psum", bufs=2, space="PSUM"))
        o_psum = ctx.enter_context(tc.tile_pool(name="o_psum", bufs=1, space="PSUM"))
        p_psum = ctx.enter_context(tc.tile_pool(name="p_psum", bufs=2, space="PSUM"))
        po_psum = ctx.enter_context(tc.tile_pool(name="po_psum", bufs=1, space="PSUM"))
        if True:
            # ---- constants to SBUF ----
            def load_w(ap_in, nm):
                t = const.tile([128, 4, DIM], BF16, name=nm, tag=nm)
                nc.sync.dma_start(out=t, in_=ap_in.rearrange("(c p) f -> p c f", p=128))
                return t

            wq, wk, wv, wo = (load_w(wq_in, "wqt"), load_w(wk_in, "wkt"),
                              load_w(wv_in, "wvt"), load_w(wo_in, "wot"))
            bq = const.tile([128, 4], F32)
            bk = const.tile([128, 4], F32)
            bo = const.tile([128, 4], F32)
            lng = const.tile([128, 4], F32)
            lnb = const.tile([128, 4], F32)
            for t, a in ((bq, bq_in), (bk, bk_in), (bo, bo_in), (lng, lng_in), (lnb, lnb_in)):
                nc.sync.dma_start(out=t, in_=a)
            bv = const.tile([1, DIM], BF16)
            nc.sync.dma_start(out=bv, in_=bv_in)
            ones = const.tile([128, 128], BF16)
            nc.sync.dma_start(out=ones, in_=ones_in)
            invd = const.tile([128, 2], BF16)
            nc.sync.dma_start(out=invd, in_=invd_in)
            magic = const.tile([1, HALF], I32)
            nc.vector.memset(magic, 0x5F3759DF)
            onesf = const.tile([1, 128], F32)
            nc.vector.memset(onesf, 1.0)

            # initial activation, as halves: f32 residual + bf16 proj copy
            x0h, x0b = [], []
            for b in range(2):
                xb = act.tile([128, 4, HALF], F32, tag=f"x0h{b}", name=f"x0h{b}")
                nc.sync.dma_start(
                    out=xb,
                    in_=xt_in.rearrange("(c p) f -> p c f", p=128)[:, :, 256 * b:256 * (b + 1)])
                x0h.append(xb)
                xbb = act.tile([128, 4, HALF], BF16, tag=f"x0b{b}", name=f"x0b{b}")
                nc.vector.tensor_copy(xbb, xb)
                x0b.append(xbb)

            # DRAM bounce buffers
            sendb = [dram.tile([NCORES, SHARD], BF16, tag=f"send{b}",
                               name=f"send{b}") for b in range(2)]
            recvb = [dram.tile([NCORES, SHARD], BF16, tag=f"recv{b}",
                               name=f"recv{b}") for b in range(2)]
            rs_d = dram.tile([1, 512], F32, tag="rs_d", name="rs_d", bufs=2)
            stat_d = [dram.tile([2, HALF], F32, tag=f"stat{b}",
                                name=f"stat{b}", bufs=2) for b in range(2)]

            def proj_T_half(x_b, w, btile, tag, pool, odt=BF16):
                """(128,4,HALF) <- relu(w^T x_b + bias), transposed output."""
                out = qkv.tile([128, 4, HALF], odt, tag=tag, name=tag)
                for pair in range(2):
                    ps = pool.tile([128, 2, HALF], F32, tag="p", name=f"ps_{tag}")
                    for i in range(2):
                        ofc = 2 * pair + i
                        for ifc in range(4):
                            nc.tensor.matmul(
                                ps[:, i, :],
                                w[:, ifc, 128 * ofc:128 * (ofc + 1)],
                                x_b[:, ifc, :],
                                start=(ifc == 0), stop=(ifc == 3))
                        nc.vector.tensor_scalar(
                            out=out[:, ofc, :], in0=ps[:, i, :],
                            scalar1=btile[:, ofc:ofc + 1], scalar2=0.0,
                            op0=OP.add, op1=OP.max)
                return out

            def proj_V_half(x_b, tag):
                """(128,2,DIM) bf16 <- relu(x_b^T wv + bv), natural layout."""
                out = qkv.tile([128, 2, DIM], BF16, tag=tag, name=tag)
                for tch in range(2):
                    ps = p_psum.tile([128, DIM], F32, tag="p", name=f"ps_{tag}{tch}")
                    for ifc in range(4):
                        nc.tensor.matmul(
                            ps,
                            x_b[:, ifc, 128 * tch:128 * (tch + 1)],
                            wv[:, ifc, :],
                            start=(ifc == 0), stop=False)
                    nc.tensor.matmul(
                        ps, ones[0:1, :], bv, start=False, stop=True)
                    nc.vector.tensor_scalar(
                        out=out[:, tch, :], in0=ps,
                        scalar1=0.0, scalar2=None, op0=OP.max)
                return out

            def exchange_half(b, qt_b, kt_b, v_b):
                sb, rb = sendb[b], recvb[b]
                # pack Q, K: 2 DMAs each (one per 64-partition half)
                for base, src in ((0, qt_b), (16384, kt_b)):
                    seg = sb[:, base:base + 16384].rearrange(
                        "(cq two) (r c) -> two r cq c", two=2, c=256)
                    for two in range(2):
                        nc.sync.dma_start(
                            out=seg[two],
                            in_=src[64 * two:64 * (two + 1), :, :])
                # pack V: 2 DMAs (one per t-chunk; 4-dim APs don't balance)
                for tc in range(2):
                    nc.sync.dma_start(
                        out=sb[:, 32768 + 8192 * tc:32768 + 8192 * (tc + 1)]
                            .rearrange("d (p j) -> p d j", p=128),
                        in_=v_b[:, tc, :].rearrange("p (d j) -> p d j", j=64))
                nc.gpsimd.collective_compute(
                    "AllToAll", OP.bypass, replica_groups=groups,
                    ins=[sb.opt()], outs=[rb.opt()])
                # unpack with 64->128 replication for row-tiled score matmuls
                qh = gath.tile([128, 8, 256], BF16, tag=f"qh{b}", name=f"qh{b}")
                kh = gath.tile([128, 8, 256], BF16, tag=f"kh{b}", name=f"kh{b}")
                for dst, base in ((qh, 0), (kh, 16384)):
                    for hh in range(2):
                        nc.sync.dma_start(
                            out=dst[64 * hh:64 * (hh + 1), :, :],
                            in_=rb[:, base:base + 16384]
                                .rearrange("s (r c) -> r s c", r=64))
                vh = gath.tile([128, 16, 65], BF16, tag=f"vh{b}", name=f"vh{b}")
                for tc2 in range(2):
                    nc.sync.dma_start(
                        out=vh[:, tc2::2, 0:64],
                        in_=rb[:, 32768 + 8192 * tc2:32768 + 8192 * (tc2 + 1)]
                            .rearrange("s (p j) -> p s j", p=128))
                nc.vector.memset(vh[:, :, 64:65], 1.0)
                return qh, kh, vh

            def attention_half(b, qh, kh, vh, fillers=()):
                """(128,4,HALF) bf16 Z^T for batch b (local Z rows).

                `fillers` are thunks (next-layer projection chunks) called
                one per score group so their PE matmuls land BETWEEN
                attention matmuls in the engine FIFOs -- the only way to
                fill the PE during the exp-bound attention stream.
                The per-column normalize (recip + z-scale) is deferred to
                the end so its DMA bounce never head-of-line-blocks the
                DVE queue between exp groups.
                """
                fill_iter = iter(fillers)
                z = qkv.tile([128, 4, HALF], BF16, tag=f"z{b}", name=f"z{b}")
                cols = []
                for j in range(4):
                    ops = o_psum.tile([65, 512], F32, tag="o", name=f"ops{b}{j}")
                    dve_groups = (2, 5) if j % 2 == 0 else (4,)
                    for g in range(8):
                        sps = s_psum.tile([128, 2, 512], F32, tag="s",
                                          name=f"sps{b}{j}{g}")
                        for u in range(2):
                            k = 2 * g + u
                            nc.tensor.matmul(
                                sps[:, u, :],
                                kh[64 * u:64 * (u + 1), k // 2,
                                   128 * (k % 2):128 * (k % 2 + 1)],
                                qh[64 * u:64 * (u + 1), 2 * j:2 * j + 2, :],
                                start=True, stop=True)
                        e = epool.tile([128, 2, 512], BF16, tag="e", name=f"e{b}{j}{g}")
                        if g in dve_groups:
                            nc.vector.tensor_scalar(
                                out=e.bitcast(I16), in0=sps,
                                scalar1=float(EXP_A), scalar2=float(EXP_B),
                                op0=OP.mult, op1=OP.add)
                        else:
                            nc.scalar.activation(e, sps, AF.Exp, scale=0.125)
                        for u in range(2):
                            k = 2 * g + u
                            nc.tensor.matmul(
                                ops, vh[:, k, :], e[:, u, :],
                                start=(k == 0), stop=(k == 15))
                        f = next(fill_iter, None)
                        if f is not None:
                            f()
                    # evacuate the single o-psum bank immediately; bounce
                    # the denominator row out for the 64-way broadcast
                    oc = small.tile([65, 512], F32, tag="oc", bufs=4, name="oc")
                    nc.scalar.activation(oc, ops, AF.Copy)
                    nc.sync.dma_start(out=rs_d, in_=oc[64:65, :])
                    dvec = small.tile([64, 512], F32, tag="dvec", bufs=4, name="dvec")
                    nc.sync.dma_start(
                        out=dvec, in_=rs_d.partition_broadcast(64)[:, 0, :])
                    cols.append((oc, dvec))
                for f in fill_iter:
                    f()
                for j, (oc, dvec) in enumerate(cols):
                    rrep = small.tile([64, 512], F32, tag="rrep", bufs=2, name="rrep")
                    nc.vector.reciprocal_approx_fast(rrep, dvec)
                    o_v = oc[0:64, :].rearrange("f (r s) -> f s r", s=8)
                    r_v = rrep.rearrange("f (r s) -> f s r", s=8)
                    for q in range(2):
                        nc.vector.tensor_tensor(
                            out=z[64 * q:64 * (q + 1), :, 64 * j:64 * (j + 1)],
                            in0=o_v[:, q::2, :],
                            in1=r_v[:, q::2, :],
                            op=OP.mult)
                return z

            def proj_fillers(b, get_x, key, pk_store):
                """Thunks that together emit QKV proj + pack + A2A + unpack
                for (layer, half) = key, sliced so one thunk fits in one
                attention score-group's PE shadow."""
                tiles = {}

                def out_tile(tag, shape):
                    if tag not in tiles:
                        tiles[tag] = qkv.tile(shape, BF16, tag=tag, name=tag)
                    return tiles[tag]

                def t_pair(w, btile, tag, pair):
                    def run():
                        x_b16 = get_x()
                        out = out_tile(tag, [128, 4, HALF])
                        ps = p_psum.tile([128, 2, HALF], F32, tag="p",
                                         name=f"ps_{tag}{pair}")
                        for i in range(2):
                            ofc = 2 * pair + i
                            for ifc in range(4):
                                nc.tensor.matmul(
                                    ps[:, i, :],
                                    w[:, ifc, 128 * ofc:128 * (ofc + 1)],
                                    x_b16[:, ifc, :],
                                    start=(ifc == 0), stop=(ifc == 3))
                            nc.vector.tensor_scalar(
                                out=out[:, ofc, :], in0=ps[:, i, :],
                                scalar1=btile[:, ofc:ofc + 1], scalar2=0.0,
                                op0=OP.add, op1=OP.max)
                    return run

                def v_chunk(tch):
                    def run():
                        x_b16 = get_x()
                        out = out_tile(f"v{b}", [128, 2, DIM])
                        ps = p_psum.tile([128, DIM], F32, tag="p",
                                         name=f"ps_v{b}{tch}")
                        for ifc in range(4):
                            nc.tensor.matmul(
                                ps,
                                x_b16[:, ifc, 128 * tch:128 * (tch + 1)],
                                wv[:, ifc, :],
                                start=(ifc == 0), stop=False)
                        nc.tensor.matmul(
                            ps, ones[0:1, :], bv, start=False, stop=True)
                        nc.vector.tensor_scalar(
                            out=out[:, tch, :], in0=ps,
                            scalar1=0.0, scalar2=None, op0=OP.max)
                    return run

                def xchg():
                    pk_store[key] = exchange_half(
                        b, tiles[f"qt{b}"], tiles[f"kt{b}"], tiles[f"v{b}"])

                return [t_pair(wq, bq, f"qt{b}", 0), t_pair(wq, bq, f"qt{b}", 1),
                        t_pair(wk, bk, f"kt{b}", 0), t_pair(wk, bk, f"kt{b}", 1),
                        v_chunk(0), v_chunk(1), xchg]

            def make_tail(m, b, z_b, resid):
                """Thunks for the post-attention tail of (m, b): O-proj,
                LN stats, variance/rsqrt chain, LN apply.  Consumed as
                fillers inside the NEXT attention phase so these PE ops
                (behind the serial LN DVE chain) never head-of-line-block
                the next attention stream in the engine FIFOs."""
                h = {}

                def t_oproj():
                    h["y"] = proj_T_half(z_b, wo, bo, f"y{b}", po_psum,
                                         odt=F32)

                def t_stats():
                    x_b = h["y"]
                    if resid is not None:
                        xr = act.tile([128, 4, HALF], F32, tag=f"xr{b}",
                                      name=f"xr{b}")
                        nc.vector.tensor_tensor(out=xr, in0=x_b, in1=resid,
                                                op=OP.add)
                        x_b = xr
                    h["x"] = x_b
                    xb16 = act.tile([128, 4, HALF], BF16, tag=f"xb{b}",
                                    name=f"xb{b}")
                    nc.vector.tensor_copy(xb16, x_b)
                    x2 = act.tile([128, 4, HALF], BF16, tag=f"x2{b}",
                                  name=f"x2{b}")
                    nc.vector.tensor_tensor(out=x2, in0=xb16, in1=xb16,
                                            op=OP.mult)
                    mu_ps = po_psum.tile([1, HALF], F32, tag="p",
                                         name=f"lnmu{b}")
                    for ifc in range(4):
                        nc.tensor.matmul(mu_ps, invd[:, 0:1], xb16[:, ifc, :],
                                         start=(ifc == 0), stop=(ifc == 3))
                    mu = small.tile([1, HALF], F32, tag="mu", bufs=2, name="mu")
                    nc.vector.tensor_copy(mu, mu_ps)
                    m2_ps = po_psum.tile([1, HALF], F32, tag="p",
                                         name=f"lnm2{b}")
                    for ifc in range(4):
                        nc.tensor.matmul(m2_ps, invd[:, 1:2], x2[:, ifc, :],
                                         start=(ifc == 0), stop=(ifc == 3))
                    h["mu"], h["m2_ps"] = mu, m2_ps

                def t_var():
                    mu, m2_ps = h["mu"], h["m2_ps"]
                    m2 = small.tile([1, HALF], F32, tag="m2", bufs=2, name="m2")
                    nc.vector.tensor_tensor(out=m2, in0=mu, in1=mu, op=OP.mult)
                    ex2 = small.tile([1, HALF], F32, tag="ex2", bufs=2,
                                     name="ex2")
                    nc.vector.tensor_tensor(out=ex2, in0=m2_ps, in1=m2,
                                            op=OP.subtract)
                    nc.vector.tensor_scalar(out=ex2, in0=ex2, scalar1=LN_EPS,
                                            scalar2=None, op0=OP.add)
                    sd = small.tile([1, HALF], F32, tag="sd", bufs=2, name="sd")
                    sdi = sd.bitcast(I32)
                    nc.vector.tensor_scalar(out=sdi, in0=ex2.bitcast(I32),
                                            scalar1=1, scalar2=None,
                                            op0=OP.logical_shift_right)
                    nc.vector.tensor_tensor(out=sdi, in0=magic, in1=sdi,
                                            op=OP.subtract)
                    for _ in range(2):
                        nc.vector.tensor_tensor(out=m2, in0=ex2, in1=sd,
                                                op=OP.mult)
                        nc.vector.tensor_tensor(out=m2, in0=m2, in1=sd,
                                                op=OP.mult)
                        nc.vector.tensor_scalar(out=m2, in0=m2, scalar1=-0.5,
                                                scalar2=1.5, op0=OP.mult,
                                                op1=OP.add)
                        nc.vector.tensor_tensor(out=sd, in0=sd, in1=m2,
                                                op=OP.mult)
                    nc.vector.tensor_tensor(out=mu, in0=mu, in1=sd, op=OP.mult)
                    srep = po_psum.tile([128, 2, HALF], F32, tag="p",
                                        name=f"srep{b}")
                    nc.tensor.matmul(srep[:, 0, :], onesf[0:1, :], sd,
                                     start=True, stop=True)
                    nc.tensor.matmul(srep[:, 1, :], onesf[0:1, :], mu,
                                     start=True, stop=True)
                    h["srep"] = srep

                def t_apply():
                    x_b, srep = h["x"], h["srep"]
                    outb = act.tile([128, 4, HALF], BF16, tag=f"lnb{b}",
                                    bufs=2, name=f"lnb{b}")
                    outf = None
                    if resid is not None:
                        outf = act.tile([128, 4, HALF], F32, tag=f"lnf{b}",
                                        bufs=2, name=f"lnf{b}")
                    for ifc in range(4):
                        t1 = small.tile([128, HALF], F32, tag=f"t1{b}", bufs=2,
                                        name=f"t1{b}")
                        nc.vector.tensor_tensor(out=t1, in0=x_b[:, ifc, :],
                                                in1=srep[:, 0, :], op=OP.mult)
                        nc.vector.tensor_tensor(out=t1, in0=t1,
                                                in1=srep[:, 1, :],
                                                op=OP.subtract)
                        nc.vector.tensor_scalar(
                            out=outb[:, ifc, :], in0=t1,
                            scalar1=lng[:, ifc:ifc + 1],
                            scalar2=lnb[:, ifc:ifc + 1],
                            op0=OP.mult, op1=OP.add)
                        if outf is not None:
                            nc.vector.tensor_scalar(
                                out=outf[:, ifc, :], in0=t1,
                                scalar1=lng[:, ifc:ifc + 1],
                                scalar2=lnb[:, ifc:ifc + 1],
                                op0=OP.mult, op1=OP.add)
                    lnb_store[(m, b)] = outb
                    if resid is not None:
                        resid_f[b] = outf
                        final_f[b] = outf

                return [t_oproj, t_stats, t_var, t_apply]

            def qkv_exchange(b, x_b16):
                qt_b = proj_T_half(x_b16, wq, bq, f"qt{b}", p_psum)
                kt_b = proj_T_half(x_b16, wk, bk, f"kt{b}", p_psum)
                v_b = proj_V_half(x_b16, f"v{b}")
                return exchange_half(b, qt_b, kt_b, v_b)

            # Flat (layer, half) software pipeline.  Each attention phase
            # consumes, as interleaved fillers: (1) the previous half's
            # full tail (O-proj + LayerNorm), then (2) the next pending
            # (layer, half)'s QKV+pack+A2A.
            NLAYERS = 2 * NITER
            resid_f = list(x0h)            # f32 residual stream (per half)
            pk = {}
            pk[(0, 0)] = qkv_exchange(0, x0b[0])
            pk[(0, 1)] = qkv_exchange(1, x0b[1])
            lnb_store = {}
            final_f = [None, None]
            pending_tail = []
            for m in range(NLAYERS):
                for b in range(2):
                    tgt = (m, 1) if b == 0 else (m + 1, 0)
                    fillers = list(pending_tail)
                    pending_tail = []
                    if tgt not in pk and tgt[0] < NLAYERS:
                        src_key = (tgt[0] - 1, tgt[1])
                        fillers += proj_fillers(
                            tgt[1], (lambda k=src_key: lnb_store[k]),
                            tgt, pk)
                    z_b = attention_half(b, *pk[(m, b)], fillers=fillers)
                    pending_tail = make_tail(
                        m, b, z_b,
                        resid_f[b] if m % 2 == 1 else None)
            for f in pending_tail:
                f()

            for b in range(2):
                nc.sync.dma_start(
                    out=out_d.rearrange("(c p) f -> p c f", p=128)[:, :, 256 * b:256 * (b + 1)],
                    in_=final_f[b])
    return nc


_NC_CACHE = None


def _get_nc():
    global _NC_CACHE
    if _NC_CACHE is None:
        nc = bacc.Bacc("TRN2", target_bir_lowering=False, debug=False,
                       num_devices=NCORES)
        _build_graph(nc)
        nc.compile()
        _NC_CACHE = nc
    return _NC_CACHE


def kernel(encoder_inputs, Wq, bq, Wk, bk, Wv, bv, Wo, bo, ln_g, ln_b,
           _trace=False, _trace_kwargs=None):
    BF = ml_dtypes.bfloat16
    x = np.asarray(encoder_inputs, dtype=np.float32)
    consts = {
        "wq": np.ascontiguousarray(np.asarray(Wq, np.float32).astype(BF)),
        "wk": np.ascontiguousarray(np.asarray(Wk, np.float32).astype(BF)),
        "wv": np.ascontiguousarray(np.asarray(Wv, np.float32).astype(BF)),
        "wo": np.ascontiguousarray(np.asarray(Wo, np.float32).astype(BF)),
        "bq": np.ascontiguousarray(np.asarray(bq, np.float32).reshape(4, 128).T),
        "bk": np.ascontiguousarray(np.asarray(bk, np.float32).reshape(4, 128).T),
        "bo": np.ascontiguousarray(np.asarray(bo, np.float32).reshape(4, 128).T),
        "bv": np.asarray(bv, np.float32).astype(BF).reshape(1, DIM),
        "lng": np.ascontiguousarray(np.asarray(ln_g, np.float32).reshape(4, 128).T),
        "lnb": np.ascontiguousarray(np.asarray(ln_b, np.float32).reshape(4, 128).T),
        "ones": np.ones((128, 128), BF),
        "invd": np.full((128, 2), 1.0 / DIM, BF),
    }
    in_maps = []
    for c in range(NCORES):
        xt = np.concatenate([x[0, 256 * c:256 * (c + 1)].T,
                             x[1, 256 * c:256 * (c + 1)].T], axis=1)
        in_maps.append({"xt": np.ascontiguousarray(xt), **consts})

    nc = _get_nc()
    res = bass_utils.run_bass_kernel_spmd(
        nc, in_maps, core_ids=list(range(NCORES)),
        trace=_trace, **(_trace_kwargs or {}))

    out = np.zeros((2, 2048, DIM), np.float32)
    for c in range(NCORES):
        r = res.results[c]["out"]
        out[0, 256 * c:256 * (c + 1)] = r[:, :256].T
        out[1, 256 * c:256 * (c + 1)] = r[:, 256:].T
    if _trace:
        kernel._last_results = res
    return out
